# revision 21
# baseline (speedup 1.0000x reference)
"""Trainium2 Bass kernel for nn_Decoder (GRU decoder with dual attention).

Strategy (8 NeuronCores, batch-parallel, zero collectives):
  - Shard batch B=64 -> 8 per core; replicate all weights.
  - Matmul operands bf16; fp32 PSUM accumulation; gates/softmax/h fp32.
  - GRU gate matmuls: stationary = transposed hidden state (th blocks),
    moving = weights, 3H split into 4 PE column-group quadrants.
    Matmuls are issued in quadrant ROUNDS (same k-tile across all 4
    quadrants back-to-back) so the 4 quadrant streams run concurrently
    (PE matmul starts are pc-monotone; round order avoids cross-quadrant
    serialization).
  - Software pipelining: the next step's W_hh rz-rounds are issued right
    after this step's score matmuls, so the PE stays busy during the
    softmax (DVE/ACT) phase.
  - Softmax uses SIGMOID instead of EXP: exp(s-max) = 1/(1-sigmoid(s-max)) - 1,
    so the scalar engine never swaps activation tables (sig/tanh share one).
  - Pose accumulation: W_out folded into attention values (p2e/p2w);
    both batch-halves (rr=0/1) accumulate into ONE [32,136] psum region
    per quadrant; transposed softmax-weight tiles have all invalid
    columns zeroed (memset once; only valid columns rewritten per step).
  - Gate bias fully folded into the wih ones-lane (rz+gi_n parts); only
    b_hh_n needs its own tiny N=256 bias round (it multiplies r).
  - Outputs: 2 strided-partition DMAs per step (batches rr*4..rr*4+4).

Layouts:
  h4  [128, 256] fp32: row 32*j+b = h[b, j*256 : (j+1)*256], b<8 valid
  th[half] [128,128] bf16: th[half][k, 32*jj+b] = h[b, jj*256+half*128+k]
  projT[k] [128, 8*192] bf16: projT[k][kk, b*192+c]: c<128 enc proj s=c,
    c in 128:192 word proj wl=c-128; h-dim k*128+kk, batch b; biases folded.
  p2e [128, 8*136]: p2e[s, b*136+o] = (W_oc @ enc_proj[s,b] + b_out)[o]
  p2w [64, 8*136]: same for words with W_ow, no bias.
  score/pose psum rows: batch b lives at row 33*(b%4)+4*(b//4).
"""

import os
import sys

sys.path.insert(0, "/opt/trn_rl_repo")

import numpy as np

S, B, E, H, O, WL, PL = 128, 64, 1024, 1024, 135, 64, 32
NCORES = 8
BL = B // NCORES          # 8 batches per core
G = 4                     # PE column-group quadrants
GH = H // G               # 256 hidden dims per quadrant
OP = 136                  # padded pose dim (135 + ones col for gi bias)
SCE, SCW = S, WL
SC = SCE + SCW            # 192 score cols per batch

_progs = {}


def _group_cols():
    """Column permutation of the 3H gate dim into G groups of [r|z|n]."""
    cols = []
    for j in range(G):
        h0 = j * GH
        cols.extend(range(h0, h0 + GH))
        cols.extend(range(H + h0, H + h0 + GH))
        cols.extend(range(2 * H + h0, 2 * H + h0 + GH))
    return np.asarray(cols)


def _body(tc, outs, ins, T, PLc):
    """Tile kernel body. ins/outs: dicts of DRAM APs."""
    from concourse import mybir
    from concourse.masks import make_identity

    nc = tc.nc
    f32 = mybir.dt.float32
    bf16 = mybir.dt.bfloat16
    SIG = mybir.ActivationFunctionType.Sigmoid
    TANH = mybir.ActivationFunctionType.Tanh
    X = mybir.AxisListType.X
    MULT = mybir.AluOpType.mult
    ADD = mybir.AluOpType.add
    SUB = mybir.AluOpType.subtract

    def mm(out, lhsT, rhs, start, stop, tp=None):
        nc.tensor.matmul(out, lhsT, rhs, start=start, stop=stop,
                         tile_position=tp, skip_group_check=True)

    import contextlib
    ctx = contextlib.ExitStack()
    with ctx:
        wp = ctx.enter_context(tc.tile_pool(name="wp", bufs=1))
        work = ctx.enter_context(tc.tile_pool(name="work", bufs=2))
        gtmp = ctx.enter_context(tc.tile_pool(name="gtmp", bufs=2))
        ps_rz = ctx.enter_context(tc.tile_pool(name="ps_rz", bufs=2, space="PSUM"))
        ps_n = ctx.enter_context(tc.tile_pool(name="ps_n", bufs=1, space="PSUM"))
        ps_sp = ctx.enter_context(tc.tile_pool(name="ps_sp", bufs=3, space="PSUM"))
        ps_tp = ctx.enter_context(tc.tile_pool(name="ps_tp", bufs=2, space="PSUM"))

        # ---------------- persistent weights ----------------
        whh = []
        for k in range(8):
            t = wp.tile([128, 3 * H], bf16, tag=f"whh{k}")
            nc.sync.dma_start(out=t, in_=ins["whh_t"][k * 128:(k + 1) * 128, :])
            whh.append(t)
        gnb = wp.tile([1, 3 * H], bf16, tag="gnb")
        nc.sync.dma_start(out=gnb, in_=ins["gbias"][:, :])
        wih0 = wp.tile([128, 3 * H], bf16, tag="wih0")
        nc.sync.dma_start(out=wih0, in_=ins["wih_t"][0:128, :])
        wih1 = wp.tile([8, 3 * H], bf16, tag="wih1")
        nc.sync.dma_start(out=wih1, in_=ins["wih_t"][128:136, :])
        woh = []
        for k in range(8):
            t = wp.tile([128, OP], bf16, tag=f"woh{k}")
            nc.sync.dma_start(out=t, in_=ins["woh_t"][k * 128:(k + 1) * 128, :])
            woh.append(t)
        posesT0 = wp.tile([128, PLc * 32], bf16, tag="posesT0")
        nc.sync.dma_start(out=posesT0, in_=ins["poses_t"][0:128, :])
        posesT1 = wp.tile([8, PLc * 32], bf16, tag="posesT1")
        nc.sync.dma_start(out=posesT1, in_=ins["poses_t"][128:136, :])

        ident = wp.tile([128, 128], f32, tag="ident")
        make_identity(nc, ident[:, :])
        ones1 = wp.tile([1, 128], bf16, tag="ones1")
        nc.vector.memset(ones1, 1.0)
        zrow = wp.tile([1, 512], bf16, tag="zrow")
        nc.vector.memset(zrow, 0.0)

        def open_group(pr_region, m, base=0):
            # dummy start=True matmul on resident operands: clears the psum
            # region without inheriting DMA waits on the first real matmul
            mm(pr_region, ones1[:, 0:m], zrow[:, 0:pr_region.shape[-1]],
               start=True, stop=False, tp=(0, base))
        bout_sb = wp.tile([1, OP], bf16, tag="bout_sb")
        nc.sync.dma_start(out=bout_sb, in_=ins["bout"][:, :])

        projT = [wp.tile([128, BL * SC], bf16, tag=f"projT{m}", name=f"projT{m}")
                 for m in range(8)]
        p2e = wp.tile([128, BL * OP], bf16, tag="p2e")
        p2w = wp.tile([64, BL * OP], bf16, tag="p2w")

        # persistent per-step tiles: invalid lanes zeroed ONCE here, only
        # valid lanes rewritten inside the loop.
        wte = [wp.tile([128, 128], bf16, tag=f"wte{rr}", name=f"wte{rr}")
               for rr in range(2)]
        wtw = [wp.tile([64, 128], bf16, tag=f"wtw{rr}", name=f"wtw{rr}")
               for rr in range(2)]
        for rr in range(2):
            nc.vector.memset(wte[rr], 0.0)
            nc.vector.memset(wtw[rr], 0.0)
        pt0 = wp.tile([128, 32], bf16, tag="pt0")
        pt1 = wp.tile([8, 32], bf16, tag="pt1")
        nc.vector.memset(pt0, 0.0)
        nc.vector.memset(pt1, 0.0)
        pose_sb2 = wp.tile([128, OP], f32, tag="pose_sb")
        nc.vector.memset(pose_sb2[:, O:OP], 1.0)
        rmask_sb = wp.tile([128, 4], f32, tag="rmask_sb")
        nc.sync.dma_start(out=rmask_sb, in_=ins["rmask"][:, :])

        # ---------------- prologue: h0 ----------------
        ehk = []
        for k in range(8):
            t = wp.tile([128, 32], bf16, tag=f"ehk{k}")
            nc.sync.dma_start(out=t, in_=ins["eht"][k * 128:(k + 1) * 128, :])
            ehk.append(t)
        eh_ones = wp.tile([1, 32], bf16, tag="eh_ones")
        nc.sync.dma_start(out=eh_ones, in_=ins["eht"][1024:1025, :])

        h0p = ps_sp.tile([128, 512], f32, tag="sp")
        for j in range(G):
            open_group(h0p[32 * j:32 * j + 32, 0:GH], 32, 32 * j)
        for k in range(9):
            kp = 128 if k < 8 else 1
            lhsT = ehk[k] if k < 8 else eh_ones
            wed = work.tile([128, H], bf16, tag="wstream", bufs=9,
                            name=f"wed{k}")
            nc.sync.dma_start(out=wed[:kp, :],
                              in_=ins["wed_t"][k * 128:k * 128 + kp, :])
            for j in range(G):
                mm(h0p[32 * j:32 * j + 32, 0:GH], lhsT,
                   wed[:kp, j * GH:(j + 1) * GH],
                   start=False, stop=(k == 8), tp=(0, 32 * j))
        h4 = gtmp.tile([128, GH], f32, tag="h4")
        nc.vector.tensor_copy(h4, h0p[:, 0:GH])

        # ---------------- prologue proj work, chunked ----------------
        # Emitted interleaved with warmup GRU steps: the independent
        # projection matmuls fill the PE during each warmup tail, keeping
        # HAM warm and hiding the warmup chain latency.
        chunks = []
        store = {}

        def c_xe(q):
            xe = []
            for k in range(9):
                kp = 128 if k < 8 else 1
                t = work.tile([128, 256], bf16, tag="xe", bufs=36,
                              name=f"xe{q}_{k}")
                nc.sync.dma_start(
                    out=t[:kp, :],
                    in_=ins["xt_enc"][k * 128:k * 128 + kp,
                                      q * 256:(q + 1) * 256],
                )
                xe.append(t)
            store[("xe", q)] = xe

        def c_enc(q, m):
            xe = store[("xe", q)]
            pr = ps_sp.tile([128, 512], f32, tag="sp", name="pr_enc")
            open_group(pr[:, 0:256], 128)
            for k in range(9):
                kp = 128 if k < 8 else 1
                wa = work.tile([128, 128], bf16, tag="wa", bufs=16,
                               name="wa_enc")
                nc.sync.dma_start(
                    out=wa[:kp, :],
                    in_=ins["watt_t"][k * 128:k * 128 + kp,
                                      m * 128:(m + 1) * 128],
                )
                mm(pr[:, 0:256], wa[:kp, :], xe[k][:kp, :],
                   start=False, stop=(k == 8))
            dst = projT[m].rearrange("p (b c) -> p b c", b=BL)
            b0 = q * 2
            nc.vector.tensor_copy(
                dst[:, b0:b0 + 2, 0:SCE],
                pr[:, 0:256].rearrange("p (b c) -> p b c", b=2),
            )

        def c_xw():
            xw0 = work.tile([128, 512], bf16, tag="xw0", bufs=1, name="xw0")
            nc.sync.dma_start(out=xw0, in_=ins["xt_word"][0:128, :])
            xw1 = work.tile([73, 512], bf16, tag="xw1", bufs=1, name="xw1")
            nc.sync.dma_start(out=xw1, in_=ins["xt_word"][128:201, :])
            store["xw"] = (xw0, xw1)

        def c_word(m):
            xw0, xw1 = store["xw"]
            pr = ps_sp.tile([128, 512], f32, tag="sp", name="pr_word")
            open_group(pr[:, 0:512], 128)
            for k in range(2):
                kp = 128 if k == 0 else 73
                ww = work.tile([128, 128], bf16, tag="wa", bufs=16,
                               name="wa_word")
                nc.sync.dma_start(
                    out=ww[:kp, :],
                    in_=ins["wwatt_t"][k * 128:k * 128 + kp,
                                       m * 128:(m + 1) * 128],
                )
                mm(pr, ww[:kp, :], (xw0 if k == 0 else xw1)[:kp, :],
                   start=False, stop=(k == 1))
            dst = projT[m].rearrange("p (b c) -> p b c", b=BL)
            nc.vector.tensor_copy(
                dst[:, :, SCE:SC],
                pr.rearrange("p (b c) -> p b c", b=BL),
            )

        def c_wocw():
            wocw = [work.tile([128, OP], bf16, tag="wocw", bufs=16,
                              name=f"wocw{k}") for k in range(8)]
            for k in range(8):
                nc.sync.dma_start(out=wocw[k],
                                  in_=ins["woc_t"][k * 128:(k + 1) * 128, :])
            store["wocw"] = wocw

        def c_p2e(b):
            wocw = store["wocw"]
            pr = ps_sp.tile([128, 512], f32, tag="sp", name="pr_p2e")
            open_group(pr[:, 0:OP], 128)
            for k in range(8):
                mm(pr[:, 0:OP], projT[k][:, b * SC:b * SC + SCE], wocw[k],
                   start=False, stop=False)
            mm(pr[:, 0:OP], ones1, bout_sb, start=False, stop=True)
            nc.vector.tensor_copy(p2e[:, b * OP:(b + 1) * OP], pr[:, 0:OP])

        def c_woww():
            woww = [work.tile([128, OP], bf16, tag="wocw", bufs=16,
                              name=f"woww{k}") for k in range(8)]
            for k in range(8):
                nc.sync.dma_start(out=woww[k],
                                  in_=ins["wow_t"][k * 128:(k + 1) * 128, :])
            store["woww"] = woww

        def c_p2w(b):
            woww = store["woww"]
            pr = ps_sp.tile([128, 512], f32, tag="sp", name="pr_p2w")
            open_group(pr[0:64, 0:OP], 64)
            for k in range(8):
                mm(pr[0:64, 0:OP], projT[k][:, b * SC + SCE:b * SC + SC],
                   woww[k], start=False, stop=(k == 7))
            nc.vector.tensor_copy(p2w[:, b * OP:(b + 1) * OP], pr[0:64, 0:OP])

        import functools
        for q in range(4):
            chunks.append(functools.partial(c_xe, q))
            for m in range(8):
                chunks.append(functools.partial(c_enc, q, m))
        chunks.append(c_xw)
        for m in range(8):
            chunks.append(functools.partial(c_word, m))
        chunks.append(c_wocw)
        for b in range(BL):
            chunks.append(functools.partial(c_p2e, b))
        chunks.append(c_woww)
        for b in range(BL):
            chunks.append(functools.partial(c_p2w, b))

        # ---------------- recurrent machinery ----------------
        def th_blk(th, k):
            return th[k % 2][:, 32 * (k // 2):32 * (k // 2) + 32]

        def emit_rz_whh(th, rz):
            # 8 quadrant-rounds of N=512 rz matmuls (W_hh k-tiles)
            for k in range(8):
                lhsT = th_blk(th, k)
                for j in range(G):
                    c0 = j * 3 * GH
                    mm(rz[32 * j:32 * j + 32, :], lhsT, whh[k][:, c0:c0 + 512],
                       start=(k == 0), stop=False, tp=(0, 32 * j))

        def emit_rz_gi(gi0, gi1, rz):
            for kk, lhsT in ((0, gi0), (1, gi1)):
                wih = wih0 if kk == 0 else wih1
                for j in range(G):
                    c0 = j * 3 * GH
                    mm(rz[32 * j:32 * j + 32, :], lhsT, wih[:, c0:c0 + 512],
                       start=False, stop=(kk == 1), tp=(0, 32 * j))

        def emit_nn_whh(th, nn_):
            # bias round (b_hh_n must be scaled by r -> kept out of wih lane)
            for j in range(G):
                c0 = j * 3 * GH
                mm(nn_[32 * j:32 * j + 32, 0:GH], ones1[:, 0:32],
                   gnb[:, c0 + 512:c0 + 768], start=True, stop=False,
                   tp=(0, 32 * j))
            for k in range(8):
                lhsT = th_blk(th, k)
                for j in range(G):
                    c0 = j * 3 * GH
                    mm(nn_[32 * j:32 * j + 32, 0:GH], lhsT,
                       whh[k][:, c0 + 512:c0 + 768],
                       start=False, stop=False, tp=(0, 32 * j))

        def emit_nn_gi(gi0, gi1, nn_):
            for kk, lhsT in ((0, gi0), (1, gi1)):
                wih = wih0 if kk == 0 else wih1
                for j in range(G):
                    c0 = j * 3 * GH
                    mm(nn_[32 * j:32 * j + 32, GH:2 * GH], lhsT,
                       wih[:, c0 + 512:c0 + 768],
                       start=(kk == 0), stop=(kk == 1), tp=(0, 32 * j))

        def emit_nn(th, gi0, gi1, nn_):
            emit_nn_whh(th, nn_)
            emit_nn_gi(gi0, gi1, nn_)

        def gru_tail(rz, nn_, h4_prev):
            """sigmoid/tanh tail; returns (h4_new, th_new)."""
            srz = gtmp.tile([128, 512], f32, tag="srz")
            nc.scalar.activation(srz[:, 0:GH], rz[:, 0:GH], SIG)
            nc.scalar.activation(srz[:, GH:2 * GH], rz[:, GH:2 * GH], SIG)
            omz = gtmp.tile([128, GH], f32, tag="omz")
            nc.scalar.activation(omz, rz[:, GH:2 * GH], SIG, scale=-1.0)
            zh = gtmp.tile([128, GH], f32, tag="zh")
            nc.gpsimd.tensor_mul(zh, srz[:, GH:2 * GH], h4_prev)
            t1 = gtmp.tile([128, GH], f32, tag="t1")
            nc.vector.tensor_mul(t1, srz[:, 0:GH], nn_[:, 0:GH])
            nc.vector.tensor_add(t1, t1, nn_[:, GH:2 * GH])
            n_sb = gtmp.tile([128, GH], f32, tag="n_sb")
            nc.scalar.activation(n_sb, t1, TANH)
            h4n = gtmp.tile([128, GH], f32, tag="h4")
            th_new = [gtmp.tile([128, 128], bf16, tag=f"th{half}",
                                name=f"th{half}")
                      for half in range(2)]
            for half in range(2):
                hs = slice(128 * half, 128 * half + 128)
                nc.vector.tensor_mul(h4n[:, hs], omz[:, hs], n_sb[:, hs])
                nc.vector.tensor_add(h4n[:, hs], h4n[:, hs], zh[:, hs])
                tpp = ps_tp.tile([128, 128], f32, tag="tp")
                nc.tensor.transpose(tpp, h4n[:, hs], ident)
                nc.vector.tensor_copy(th_new[half], tpp)
            return h4n, th_new

        # ---------------- warmup over previous poses ----------------
        # initial transpose of h0
        th = [gtmp.tile([128, 128], bf16, tag=f"th{half}", name=f"th{half}")
              for half in range(2)]
        for half in range(2):
            tpp = ps_tp.tile([128, 128], f32, tag="tp")
            nc.tensor.transpose(tpp, h4[:, 128 * half:128 * half + 128], ident)
            nc.vector.tensor_copy(th[half], tpp)

        for t in range(PLc):
            gi0 = posesT0[:, t * 32:(t + 1) * 32]
            gi1 = posesT1[:, t * 32:(t + 1) * 32]
            rz = ps_rz.tile([128, 512], f32, tag="rz")
            nn_ = ps_n.tile([128, 512], f32, tag="nn")
            emit_rz_whh(th, rz)
            emit_rz_gi(gi0, gi1, rz)
            emit_nn(th, gi0, gi1, nn_)
            h4, th = gru_tail(rz, nn_, h4)
            for _ in range(2):
                if chunks:
                    chunks.pop(0)()
        while chunks:
            chunks.pop(0)()

        # ---------------- main loop ----------------
        poseT0 = posesT0[:, (PLc - 1) * 32:PLc * 32]
        poseT1 = posesT1[:, (PLc - 1) * 32:PLc * 32]
        poses_dram = outs["poses"]

        # pre-issue step 0's gru matmuls (pipelined pattern)
        rz_cur = ps_rz.tile([128, 512], f32, tag="rz")
        nn_cur = ps_n.tile([128, 512], f32, tag="nn")
        emit_rz_whh(th, rz_cur)
        emit_nn_whh(th, nn_cur)
        emit_rz_gi(poseT0, poseT1, rz_cur)

        for t in range(T):
            emit_nn_gi(poseT0, poseT1, nn_cur)
            h4, th = gru_tail(rz_cur, nn_cur, h4)

            # scores (k-outer rounds for quadrant concurrency)
            sc = ps_sp.tile([128, 512], f32, tag="sp")
            for j in range(G):
                open_group(sc[32 * j:32 * j + 32, 0:2 * SC], 32, 32 * j)
            for k in range(8):
                lhsT = th_blk(th, k)
                for b in range(BL):
                    j, rr = b % 4, b // 4
                    mm(sc[32 * j:32 * j + 32, rr * SC:(rr + 1) * SC],
                       lhsT, projT[k][:, b * SC:(b + 1) * SC],
                       start=False, stop=(k == 7), tp=(0, 32 * j))

            # next step's W_hh rz+nn rounds: fills the PE during softmax
            if t < T - 1:
                rz_nxt = ps_rz.tile([128, 512], f32, tag="rz")
                nn_nxt = ps_n.tile([128, 512], f32, tag="nn")
                emit_rz_whh(th, rz_nxt)
                emit_nn_whh(th, nn_nxt)

            # softmax via sigmoid: exp(s - max) = 1/(1 - sig(s - max)) - 1
            nmax = gtmp.tile([128, 4], f32, tag="nmax")
            sig = gtmp.tile([128, 2 * SC], f32, tag="sig")
            rcp = gtmp.tile([128, 2 * SC], f32, tag="rcp")
            sumr = gtmp.tile([128, 4], f32, tag="sumr")
            rinv = gtmp.tile([128, 4], f32, tag="rinv")
            w_sb = gtmp.tile([128, 2 * SC], f32, tag="w_sb")
            # col layout in nmax/sumr/rinv: enc rr -> col rr, word rr -> 2+rr
            sc2 = sc[:, 0:2 * SC].rearrange("p (r c) -> p r c", r=2)
            nc.vector.reduce_max(out=nmax[:, 0:2], in_=sc2[:, :, 0:SCE],
                                 axis=X, negate=True)
            nc.vector.reduce_max(out=nmax[:, 2:4], in_=sc2[:, :, SCE:SC],
                                 axis=X, negate=True)
            for rr in range(2):
                nc.scalar.activation(sig[:, rr * SC:rr * SC + SCE],
                                     sc[:, rr * SC:rr * SC + SCE],
                                     SIG, bias=nmax[:, rr:rr + 1])
                nc.scalar.activation(sig[:, rr * SC + SCE:(rr + 1) * SC],
                                     sc[:, rr * SC + SCE:(rr + 1) * SC],
                                     SIG, bias=nmax[:, 2 + rr:3 + rr])
            nc.vector.tensor_scalar(rcp, sig, -1.0, 1.0, MULT, ADD)
            nc.vector.reciprocal_approx_fast(out=rcp, in_=rcp)
            rcp2 = rcp.rearrange("p (r c) -> p r c", r=2)
            nc.vector.reduce_sum(out=sumr[:, 0:2], in_=rcp2[:, :, 0:SCE],
                                 axis=X)
            nc.vector.reduce_sum(out=sumr[:, 2:4], in_=rcp2[:, :, SCE:SC],
                                 axis=X)
            nc.vector.tensor_scalar(rinv[:, 0:2], sumr[:, 0:2],
                                    float(SCE), None, SUB)
            nc.vector.tensor_scalar(rinv[:, 2:4], sumr[:, 2:4],
                                    float(SCW), None, SUB)
            nc.vector.reciprocal(rinv, rinv)
            nc.vector.tensor_scalar_sub(w_sb, rcp, 1.0)
            # per-row normalizers: rins col0 = 1/sum_enc, col1 = 1/sum_word
            # (row 33j+4rr is batch 4rr+j; rmask picks the rr-matching col)
            rins = gtmp.tile([128, 2], f32, tag="rins")
            scrap = gtmp.tile([128, 4], f32, tag="scrap")
            nc.vector.tensor_mul(scrap, rinv, rmask_sb)
            nc.vector.tensor_add(rins[:, 0:1], scrap[:, 0:1], scrap[:, 1:2])
            nc.vector.tensor_add(rins[:, 1:2], scrap[:, 2:3], scrap[:, 3:4])

            # transpose softmax weights -> zero-padded bf16 stationaries
            for rr in range(2):
                tpp = ps_tp.tile([128, 128], f32, tag="tp")
                nc.tensor.transpose(tpp, w_sb[:, rr * SC:rr * SC + SCE], ident)
                nc.vector.tensor_copy(wte[rr][:, 4 * rr:4 * rr + 100:33],
                                      tpp[:, 4 * rr:4 * rr + 100:33])
                tp2 = ps_tp.tile([128, 128], f32, tag="tp")
                nc.tensor.transpose(tp2[0:64, :],
                                    w_sb[:, rr * SC + SCE:rr * SC + SC], ident)
                nc.vector.tensor_copy(wtw[rr][:, 4 * rr:4 * rr + 100:33],
                                      tp2[0:64, 4 * rr:4 * rr + 100:33])

            # pose accumulation: one [32,136] region per quadrant; both
            # rr-halves accumulate (invalid wte/wtw columns are zero)
            pp = ps_sp.tile([128, 512], f32, tag="sp")
            for j in range(G):
                open_group(pp[32 * j:32 * j + 32, 0:3 * OP], 32, 32 * j)
            for k in range(8):
                lhsT = th_blk(th, k)
                for j in range(G):
                    mm(pp[32 * j:32 * j + 32, 0:OP], lhsT, woh[k],
                       start=False, stop=False, tp=(0, 32 * j))
            for rr in range(2):
                for j in range(G):
                    b = 4 * rr + j
                    mm(pp[32 * j:32 * j + 32, OP:2 * OP],
                       wte[rr][:, 32 * j:32 * j + 32],
                       p2e[:, b * OP:(b + 1) * OP],
                       start=False, stop=False, tp=(0, 32 * j))
            for rr in range(2):
                for j in range(G):
                    b = 4 * rr + j
                    mm(pp[32 * j:32 * j + 32, 2 * OP:3 * OP],
                       wtw[rr][0:64, 32 * j:32 * j + 32],
                       p2w[0:64, b * OP:(b + 1) * OP],
                       start=False, stop=(rr == 1), tp=(0, 32 * j))

            # combine: pose = Woh.h + enc_fold/sum_e + word_fold/sum_w
            tB = gtmp.tile([128, O], f32, tag="tB")
            nc.vector.tensor_scalar_mul(tB, pp[:, OP:OP + O], rins[:, 0:1])
            tC = gtmp.tile([128, O], f32, tag="tC")
            nc.vector.tensor_scalar_mul(tC, pp[:, 2 * OP:2 * OP + O],
                                        rins[:, 1:2])
            nc.vector.tensor_add(tB, tB, pp[:, 0:O])
            nc.vector.tensor_add(pose_sb2[:, 0:O], tB, tC)
            for rr in range(2):
                nc.sync.dma_start(
                    out=poses_dram[t, 4 * rr:4 * rr + 4, :],
                    in_=pose_sb2[4 * rr:4 * rr + 100:33, 0:O],
                )
            if t == T - 1:
                break
            # pose -> transposed gi stationaries (valid cols only; rest
            # stay zero from the prologue memset)
            tpp = ps_tp.tile([128, 128], f32, tag="tp")
            nc.tensor.transpose(tpp, pose_sb2[:, 0:128], ident)
            tp2 = ps_tp.tile([128, 128], f32, tag="tp")
            nc.tensor.transpose(tp2[0:8, :], pose_sb2[:, 128:OP], ident)
            for rr in range(2):
                nc.vector.tensor_copy(pt0[:, 4 * rr:4 * rr + 4],
                                      tpp[:, 4 * rr:4 * rr + 100:33])
                nc.vector.tensor_copy(pt1[:, 4 * rr:4 * rr + 4],
                                      tp2[0:8, 4 * rr:4 * rr + 100:33])
            poseT0, poseT1 = pt0, pt1
            # finish next step's rz accumulation now that poseT is ready
            emit_rz_gi(poseT0, poseT1, rz_nxt)
            rz_cur, nn_cur = rz_nxt, nn_nxt


def _build(T, PLc):
    import concourse.tile as tile
    from concourse import bacc, mybir

    f32 = mybir.dt.float32
    bf16 = mybir.dt.bfloat16
    nc = bacc.Bacc("TRN2", target_bir_lowering=False, debug=False,
                   num_devices=NCORES)
    ins = {}

    def di(name, shape, dt=bf16):
        ins[name] = nc.dram_tensor(name, list(shape), dt,
                                   kind="ExternalInput").ap()

    di("xt_enc", (E + 1, BL * S))
    di("xt_word", (201, BL * WL))
    di("eht", (E + 1, 32))
    di("poses_t", (136, PLc * 32))
    di("whh_t", (H, 3 * H))
    di("gbias", (1, 3 * H))
    di("wih_t", (136, 3 * H))
    di("woh_t", (H, OP))
    di("woc_t", (H, OP))
    di("wow_t", (H, OP))
    di("bout", (1, OP))
    di("watt_t", (E + 1, H))
    di("rmask", (128, 4), f32)
    di("wwatt_t", (201, H))
    di("wed_t", (E + 1, H))
    outs = {"poses": nc.dram_tensor("poses", [T, BL, O], f32,
                                    kind="ExternalOutput").ap()}
    with tile.TileContext(nc) as tc:
        _body(tc, outs, ins, T, PLc)
    nc.compile()
    return nc


def _host_prep(inputs, PLc=PL):
    """Per-core input maps (host transposes + weight prep), bf16."""
    import ml_dtypes
    bf = ml_dtypes.bfloat16

    enc = np.asarray(inputs["encoder_states"], np.float32)
    ehid = np.asarray(inputs["encoder_hidden"], np.float32)
    pp = np.asarray(inputs["previous_poses"], np.float32)
    words = np.asarray(inputs["words"], np.float32)
    W_ed, b_ed = np.asarray(inputs["W_ed"], np.float32), np.asarray(inputs["b_ed"], np.float32)
    W_att, b_att = np.asarray(inputs["W_att"], np.float32), np.asarray(inputs["b_att"], np.float32)
    W_watt, b_watt = np.asarray(inputs["W_watt"], np.float32), np.asarray(inputs["b_watt"], np.float32)
    W_ih, W_hh = np.asarray(inputs["W_ih"], np.float32), np.asarray(inputs["W_hh"], np.float32)
    b_ih, b_hh = np.asarray(inputs["b_ih"], np.float32), np.asarray(inputs["b_hh"], np.float32)
    W_out, b_out = np.asarray(inputs["W_out"], np.float32), np.asarray(inputs["b_out"], np.float32)

    gc = _group_cols()
    bihg = b_ih[gc]
    bhhg = b_hh[gc]
    # wih ones-lane bias: full (b_ih+b_hh) in rz slots, b_ih only in n slots
    wih_bias = bihg + bhhg
    # b_hh_n rides its own bias round (it is scaled by r)
    gnb = np.zeros(3 * H, np.float32)
    for j in range(G):
        c0 = j * 3 * GH
        wih_bias[c0 + 512:c0 + 768] = bihg[c0 + 512:c0 + 768]
        gnb[c0 + 512:c0 + 768] = bhhg[c0 + 512:c0 + 768]

    whh_t = W_hh.T[:, gc]
    wih_t = np.zeros((136, 3 * H), np.float32)
    wih_t[:O] = W_ih.T[:, gc]
    wih_t[O] = wih_bias

    woh_t = np.zeros((H, OP), np.float32)
    woh_t[:, :O] = W_out[:, :H].T
    woc_t = np.zeros((H, OP), np.float32)
    woc_t[:, :O] = W_out[:, H:2 * H].T
    wow_t = np.zeros((H, OP), np.float32)
    wow_t[:, :O] = W_out[:, 2 * H:].T
    bout = np.zeros((1, OP), np.float32)
    bout[0, :O] = b_out

    watt_t = np.concatenate([W_att.T, b_att[None, :]], 0)
    wwatt_t = np.concatenate([W_watt.T, b_watt[None, :]], 0)
    wed_t = np.concatenate([W_ed.T, b_ed[None, :]], 0)

    rmask = np.zeros((128, 4), np.float32)
    for j in range(4):
        for rr in range(2):
            row = 33 * j + 4 * rr
            rmask[row, rr] = 1.0
            rmask[row, 2 + rr] = 1.0

    shared = dict(whh_t=whh_t, gbias=gnb[None, :], wih_t=wih_t, woh_t=woh_t,
                  woc_t=woc_t, wow_t=wow_t, bout=bout, watt_t=watt_t,
                  wwatt_t=wwatt_t, wed_t=wed_t)
    shared = {k: np.ascontiguousarray(v.astype(bf)) for k, v in shared.items()}
    shared["rmask"] = np.ascontiguousarray(rmask)

    in_maps = []
    for c in range(NCORES):
        bs = slice(c * BL, (c + 1) * BL)
        xt_enc = np.zeros((E + 1, BL * S), np.float32)
        xt_enc[:E] = np.transpose(enc[:, bs, :], (2, 1, 0)).reshape(E, BL * S)
        xt_enc[E] = 1.0
        xt_word = np.zeros((201, BL * WL), np.float32)
        xt_word[:200] = np.transpose(words[:, bs, :], (2, 1, 0)).reshape(200, BL * WL)
        xt_word[200] = 1.0
        eh = np.transpose(ehid[:, bs, :], (1, 0, 2)).reshape(BL, E)
        eht = np.zeros((E + 1, 32), np.float32)
        eht[:E, :BL] = eh.T
        eht[E, :BL] = 1.0
        poses_t = np.zeros((136, PLc, 32), np.float32)
        poses_t[:O, :, :BL] = np.transpose(pp[:, bs, :], (2, 0, 1))
        poses_t[O, :, :BL] = 1.0
        poses_t = poses_t.reshape(136, PLc * 32)
        m = dict(xt_enc=xt_enc, xt_word=xt_word, eht=eht, poses_t=poses_t)
        m = {k: np.ascontiguousarray(v.astype(bf)) for k, v in m.items()}
        m.update(shared)
        in_maps.append(m)
    return in_maps


def kernel(**inputs):
    from concourse.bass_utils import run_bass_kernel_spmd

    T = int(inputs["real_poses_len"])
    PLc = int(inputs["previous_poses"].shape[0])
    key = (T, PLc)
    if key not in _progs:
        _progs[key] = _build(T, PLc)
    nc = _progs[key]
    in_maps = _host_prep(inputs, PLc)
    trace = bool(int(os.environ.get("KERNEL_TRACE", "0")))
    res = run_bass_kernel_spmd(nc, in_maps, core_ids=list(range(NCORES)),
                               trace=trace)
    if trace:
        kernel.last_exec_time_ns = res.exec_time_ns
        kernel.last_mean_exec_time_ns = res.mean_exec_time_ns
    out = np.concatenate([res.results[c]["poses"] for c in range(NCORES)], axis=1)
    return out.astype(np.float32)


# revision 22
# speedup vs baseline: 1.0335x; 1.0335x over previous
"""Trainium2 Bass kernel for nn_Decoder (GRU decoder with dual attention).

Strategy (8 NeuronCores, batch-parallel, zero collectives):
  - Shard batch B=64 -> 8 per core; replicate all weights.
  - Matmul operands bf16; fp32 PSUM accumulation; gates/softmax/h fp32.
  - GRU gate matmuls: stationary = transposed hidden state (th blocks),
    moving = weights, 3H split into 4 PE column-group quadrants.
    Matmuls are issued in quadrant ROUNDS (same k-tile across all 4
    quadrants back-to-back) so the 4 quadrant streams run concurrently
    (PE matmul starts are pc-monotone; round order avoids cross-quadrant
    serialization).
  - Software pipelining: the next step's W_hh rz-rounds are issued right
    after this step's score matmuls, so the PE stays busy during the
    softmax (DVE/ACT) phase.
  - Softmax uses SIGMOID instead of EXP: exp(s-max) = 1/(1-sigmoid(s-max)) - 1,
    so the scalar engine never swaps activation tables (sig/tanh share one).
  - Pose accumulation: W_out folded into attention values (p2e/p2w);
    both batch-halves (rr=0/1) accumulate into ONE [32,136] psum region
    per quadrant; transposed softmax-weight tiles have all invalid
    columns zeroed (memset once; only valid columns rewritten per step).
  - Gate bias fully folded into the wih ones-lane (rz+gi_n parts); only
    b_hh_n needs its own tiny N=256 bias round (it multiplies r).
  - Outputs: 2 strided-partition DMAs per step (batches rr*4..rr*4+4).

Layouts:
  h4  [128, 256] fp32: row 32*j+b = h[b, j*256 : (j+1)*256], b<8 valid
  th[half] [128,128] bf16: th[half][k, 32*jj+b] = h[b, jj*256+half*128+k]
  projT[k] [128, 8*192] bf16: projT[k][kk, b*192+c]: c<128 enc proj s=c,
    c in 128:192 word proj wl=c-128; h-dim k*128+kk, batch b; biases folded.
  p2e [128, 8*136]: p2e[s, b*136+o] = (W_oc @ enc_proj[s,b] + b_out)[o]
  p2w [64, 8*136]: same for words with W_ow, no bias.
  score/pose psum rows: batch b lives at row 33*(b%4)+4*(b//4).
"""

import os
import sys

sys.path.insert(0, "/opt/trn_rl_repo")

import numpy as np

S, B, E, H, O, WL, PL = 128, 64, 1024, 1024, 135, 64, 32
NCORES = 8
BL = B // NCORES          # 8 batches per core
G = 4                     # PE column-group quadrants
GH = H // G               # 256 hidden dims per quadrant
OP = 136                  # padded pose dim (135 + ones col for gi bias)
SCE, SCW = S, WL
SC = SCE + SCW            # 192 score cols per batch

_progs = {}


def _group_cols():
    """Column permutation of the 3H gate dim into G groups of [r|z|n]."""
    cols = []
    for j in range(G):
        h0 = j * GH
        cols.extend(range(h0, h0 + GH))
        cols.extend(range(H + h0, H + h0 + GH))
        cols.extend(range(2 * H + h0, 2 * H + h0 + GH))
    return np.asarray(cols)


def _body(tc, outs, ins, T, PLc):
    """Tile kernel body. ins/outs: dicts of DRAM APs."""
    from concourse import mybir
    from concourse.masks import make_identity

    nc = tc.nc
    f32 = mybir.dt.float32
    bf16 = mybir.dt.bfloat16
    SIG = mybir.ActivationFunctionType.Sigmoid
    TANH = mybir.ActivationFunctionType.Tanh
    X = mybir.AxisListType.X
    MULT = mybir.AluOpType.mult
    ADD = mybir.AluOpType.add
    SUB = mybir.AluOpType.subtract

    def mm(out, lhsT, rhs, start, stop, tp=None):
        nc.tensor.matmul(out, lhsT, rhs, start=start, stop=stop,
                         tile_position=tp, skip_group_check=True)

    import contextlib
    ctx = contextlib.ExitStack()
    with ctx:
        wp = ctx.enter_context(tc.tile_pool(name="wp", bufs=1))
        work = ctx.enter_context(tc.tile_pool(name="work", bufs=2))
        gtmp = ctx.enter_context(tc.tile_pool(name="gtmp", bufs=2))
        ps_rz = ctx.enter_context(tc.tile_pool(name="ps_rz", bufs=2, space="PSUM"))
        ps_n = ctx.enter_context(tc.tile_pool(name="ps_n", bufs=1, space="PSUM"))
        ps_sp = ctx.enter_context(tc.tile_pool(name="ps_sp", bufs=3, space="PSUM"))
        ps_tp = ctx.enter_context(tc.tile_pool(name="ps_tp", bufs=2, space="PSUM"))

        # ---------------- persistent weights ----------------
        whh = []
        for k in range(8):
            t = wp.tile([128, 3 * H], bf16, tag=f"whh{k}")
            nc.sync.dma_start(out=t, in_=ins["whh_t"][k * 128:(k + 1) * 128, :])
            whh.append(t)
        gnb = wp.tile([1, 3 * H], bf16, tag="gnb")
        nc.sync.dma_start(out=gnb, in_=ins["gbias"][:, :])
        wih0 = wp.tile([128, 3 * H], bf16, tag="wih0")
        nc.sync.dma_start(out=wih0, in_=ins["wih_t"][0:128, :])
        wih1 = wp.tile([8, 3 * H], bf16, tag="wih1")
        nc.sync.dma_start(out=wih1, in_=ins["wih_t"][128:136, :])
        woh = []
        for k in range(8):
            t = wp.tile([128, OP], bf16, tag=f"woh{k}")
            nc.sync.dma_start(out=t, in_=ins["woh_t"][k * 128:(k + 1) * 128, :])
            woh.append(t)
        posesT0 = wp.tile([128, PLc * 32], bf16, tag="posesT0")
        nc.sync.dma_start(out=posesT0, in_=ins["poses_t"][0:128, :])
        posesT1 = wp.tile([8, PLc * 32], bf16, tag="posesT1")
        nc.sync.dma_start(out=posesT1, in_=ins["poses_t"][128:136, :])

        ident = wp.tile([128, 128], f32, tag="ident")
        make_identity(nc, ident[:, :])
        ones1 = wp.tile([1, 128], bf16, tag="ones1")
        nc.vector.memset(ones1, 1.0)
        zrow = wp.tile([1, 512], bf16, tag="zrow")
        nc.vector.memset(zrow, 0.0)

        def open_group(pr_region, m, base=0):
            # dummy start=True matmul on resident operands: clears the psum
            # region without inheriting DMA waits on the first real matmul
            mm(pr_region, ones1[:, 0:m], zrow[:, 0:pr_region.shape[-1]],
               start=True, stop=False, tp=(0, base))
        bout_sb = wp.tile([1, OP], bf16, tag="bout_sb")
        nc.sync.dma_start(out=bout_sb, in_=ins["bout"][:, :])

        projT = [wp.tile([128, BL * SC], bf16, tag=f"projT{m}", name=f"projT{m}")
                 for m in range(8)]
        p2e = wp.tile([128, BL * OP], bf16, tag="p2e")
        p2w = wp.tile([64, BL * OP], bf16, tag="p2w")

        # persistent per-step tiles: invalid lanes zeroed ONCE here, only
        # valid lanes rewritten inside the loop.
        wte = [wp.tile([128, 128], bf16, tag=f"wte{rr}", name=f"wte{rr}")
               for rr in range(2)]
        wtw = [wp.tile([64, 128], bf16, tag=f"wtw{rr}", name=f"wtw{rr}")
               for rr in range(2)]
        for rr in range(2):
            nc.vector.memset(wte[rr], 0.0)
            nc.vector.memset(wtw[rr], 0.0)
        pt0 = wp.tile([128, 32], bf16, tag="pt0")
        pt1 = wp.tile([8, 32], bf16, tag="pt1")
        nc.vector.memset(pt0, 0.0)
        nc.vector.memset(pt1, 0.0)
        pose_sb2 = wp.tile([128, OP], f32, tag="pose_sb")
        nc.vector.memset(pose_sb2[:, O:OP], 1.0)

        # ---------------- prologue: h0 ----------------
        ehk = []
        for k in range(8):
            t = wp.tile([128, 32], bf16, tag=f"ehk{k}")
            nc.sync.dma_start(out=t, in_=ins["eht"][k * 128:(k + 1) * 128, :])
            ehk.append(t)
        eh_ones = wp.tile([1, 32], bf16, tag="eh_ones")
        nc.sync.dma_start(out=eh_ones, in_=ins["eht"][1024:1025, :])

        h0p = ps_sp.tile([128, 512], f32, tag="sp")
        for j in range(G):
            open_group(h0p[32 * j:32 * j + 32, 0:GH], 32, 32 * j)
        for k in range(9):
            kp = 128 if k < 8 else 1
            lhsT = ehk[k] if k < 8 else eh_ones
            wed = work.tile([128, H], bf16, tag="wstream", bufs=9,
                            name=f"wed{k}")
            nc.sync.dma_start(out=wed[:kp, :],
                              in_=ins["wed_t"][k * 128:k * 128 + kp, :])
            for j in range(G):
                mm(h0p[32 * j:32 * j + 32, 0:GH], lhsT,
                   wed[:kp, j * GH:(j + 1) * GH],
                   start=False, stop=(k == 8), tp=(0, 32 * j))
        h4 = gtmp.tile([128, GH], f32, tag="h4")
        nc.vector.tensor_copy(h4, h0p[:, 0:GH])

        # ---------------- prologue proj work, chunked ----------------
        # Emitted interleaved with warmup GRU steps: the independent
        # projection matmuls fill the PE during each warmup tail, keeping
        # HAM warm and hiding the warmup chain latency.
        chunks = []
        store = {}

        def c_xe(q):
            xe = []
            for k in range(9):
                kp = 128 if k < 8 else 1
                t = work.tile([128, 256], bf16, tag="xe", bufs=36,
                              name=f"xe{q}_{k}")
                nc.sync.dma_start(
                    out=t[:kp, :],
                    in_=ins["xt_enc"][k * 128:k * 128 + kp,
                                      q * 256:(q + 1) * 256],
                )
                xe.append(t)
            store[("xe", q)] = xe

        def c_enc(q, m):
            xe = store[("xe", q)]
            pr = ps_sp.tile([128, 512], f32, tag="sp", name="pr_enc")
            open_group(pr[:, 0:256], 128)
            for k in range(9):
                kp = 128 if k < 8 else 1
                wa = work.tile([128, 128], bf16, tag="wa", bufs=16,
                               name="wa_enc")
                nc.sync.dma_start(
                    out=wa[:kp, :],
                    in_=ins["watt_t"][k * 128:k * 128 + kp,
                                      m * 128:(m + 1) * 128],
                )
                mm(pr[:, 0:256], wa[:kp, :], xe[k][:kp, :],
                   start=False, stop=(k == 8))
            dst = projT[m].rearrange("p (b c) -> p b c", b=BL)
            b0 = q * 2
            nc.vector.tensor_copy(
                dst[:, b0:b0 + 2, 0:SCE],
                pr[:, 0:256].rearrange("p (b c) -> p b c", b=2),
            )

        def c_xw():
            xw0 = work.tile([128, 512], bf16, tag="xw0", bufs=1, name="xw0")
            nc.sync.dma_start(out=xw0, in_=ins["xt_word"][0:128, :])
            xw1 = work.tile([73, 512], bf16, tag="xw1", bufs=1, name="xw1")
            nc.sync.dma_start(out=xw1, in_=ins["xt_word"][128:201, :])
            store["xw"] = (xw0, xw1)

        def c_word(m):
            xw0, xw1 = store["xw"]
            pr = ps_sp.tile([128, 512], f32, tag="sp", name="pr_word")
            open_group(pr[:, 0:512], 128)
            for k in range(2):
                kp = 128 if k == 0 else 73
                ww = work.tile([128, 128], bf16, tag="wa", bufs=16,
                               name="wa_word")
                nc.sync.dma_start(
                    out=ww[:kp, :],
                    in_=ins["wwatt_t"][k * 128:k * 128 + kp,
                                       m * 128:(m + 1) * 128],
                )
                mm(pr, ww[:kp, :], (xw0 if k == 0 else xw1)[:kp, :],
                   start=False, stop=(k == 1))
            dst = projT[m].rearrange("p (b c) -> p b c", b=BL)
            nc.vector.tensor_copy(
                dst[:, :, SCE:SC],
                pr.rearrange("p (b c) -> p b c", b=BL),
            )

        def c_wocw():
            wocw = [work.tile([128, OP], bf16, tag="wocw", bufs=16,
                              name=f"wocw{k}") for k in range(8)]
            for k in range(8):
                nc.sync.dma_start(out=wocw[k],
                                  in_=ins["woc_t"][k * 128:(k + 1) * 128, :])
            store["wocw"] = wocw

        def c_p2e(b):
            wocw = store["wocw"]
            pr = ps_sp.tile([128, 512], f32, tag="sp", name="pr_p2e")
            open_group(pr[:, 0:OP], 128)
            for k in range(8):
                mm(pr[:, 0:OP], projT[k][:, b * SC:b * SC + SCE], wocw[k],
                   start=False, stop=False)
            mm(pr[:, 0:OP], ones1, bout_sb, start=False, stop=True)
            nc.vector.tensor_copy(p2e[:, b * OP:(b + 1) * OP], pr[:, 0:OP])

        def c_woww():
            woww = [work.tile([128, OP], bf16, tag="wocw", bufs=16,
                              name=f"woww{k}") for k in range(8)]
            for k in range(8):
                nc.sync.dma_start(out=woww[k],
                                  in_=ins["wow_t"][k * 128:(k + 1) * 128, :])
            store["woww"] = woww

        def c_p2w(b):
            woww = store["woww"]
            pr = ps_sp.tile([128, 512], f32, tag="sp", name="pr_p2w")
            open_group(pr[0:64, 0:OP], 64)
            for k in range(8):
                mm(pr[0:64, 0:OP], projT[k][:, b * SC + SCE:b * SC + SC],
                   woww[k], start=False, stop=(k == 7))
            nc.vector.tensor_copy(p2w[:, b * OP:(b + 1) * OP], pr[0:64, 0:OP])

        import functools
        for q in range(4):
            chunks.append(functools.partial(c_xe, q))
            for m in range(8):
                chunks.append(functools.partial(c_enc, q, m))
        chunks.append(c_xw)
        for m in range(8):
            chunks.append(functools.partial(c_word, m))
        chunks.append(c_wocw)
        for b in range(BL):
            chunks.append(functools.partial(c_p2e, b))
        chunks.append(c_woww)
        for b in range(BL):
            chunks.append(functools.partial(c_p2w, b))

        # ---------------- recurrent machinery ----------------
        def th_blk(th, k):
            return th[k % 2][:, 32 * (k // 2):32 * (k // 2) + 32]

        def emit_rz_whh(th, rz):
            # 8 quadrant-rounds of N=512 rz matmuls (W_hh k-tiles)
            for k in range(8):
                lhsT = th_blk(th, k)
                for j in range(G):
                    c0 = j * 3 * GH
                    mm(rz[32 * j:32 * j + 32, :], lhsT, whh[k][:, c0:c0 + 512],
                       start=(k == 0), stop=False, tp=(0, 32 * j))

        def emit_rz_gi(gi0, gi1, rz):
            for kk, lhsT in ((0, gi0), (1, gi1)):
                wih = wih0 if kk == 0 else wih1
                for j in range(G):
                    c0 = j * 3 * GH
                    mm(rz[32 * j:32 * j + 32, :], lhsT, wih[:, c0:c0 + 512],
                       start=False, stop=(kk == 1), tp=(0, 32 * j))

        def emit_nn_whh(th, nn_):
            # bias round (b_hh_n must be scaled by r -> kept out of wih lane)
            for j in range(G):
                c0 = j * 3 * GH
                mm(nn_[32 * j:32 * j + 32, 0:GH], ones1[:, 0:32],
                   gnb[:, c0 + 512:c0 + 768], start=True, stop=False,
                   tp=(0, 32 * j))
            for k in range(8):
                lhsT = th_blk(th, k)
                for j in range(G):
                    c0 = j * 3 * GH
                    mm(nn_[32 * j:32 * j + 32, 0:GH], lhsT,
                       whh[k][:, c0 + 512:c0 + 768],
                       start=False, stop=False, tp=(0, 32 * j))

        def emit_nn_gi(gi0, gi1, nn_):
            for kk, lhsT in ((0, gi0), (1, gi1)):
                wih = wih0 if kk == 0 else wih1
                for j in range(G):
                    c0 = j * 3 * GH
                    mm(nn_[32 * j:32 * j + 32, GH:2 * GH], lhsT,
                       wih[:, c0 + 512:c0 + 768],
                       start=(kk == 0), stop=(kk == 1), tp=(0, 32 * j))

        def emit_nn(th, gi0, gi1, nn_):
            emit_nn_whh(th, nn_)
            emit_nn_gi(gi0, gi1, nn_)

        def gru_tail(rz, nn_, h4_prev):
            """sigmoid/tanh tail; returns (h4_new, th_new)."""
            srz = gtmp.tile([128, 512], f32, tag="srz")
            nc.scalar.activation(srz[:, 0:GH], rz[:, 0:GH], SIG)
            nc.scalar.activation(srz[:, GH:2 * GH], rz[:, GH:2 * GH], SIG)
            omz = gtmp.tile([128, GH], f32, tag="omz")
            nc.scalar.activation(omz, rz[:, GH:2 * GH], SIG, scale=-1.0)
            zh = gtmp.tile([128, GH], f32, tag="zh")
            nc.gpsimd.tensor_mul(zh, srz[:, GH:2 * GH], h4_prev)
            t1 = gtmp.tile([128, GH], f32, tag="t1")
            nc.vector.tensor_mul(t1, srz[:, 0:GH], nn_[:, 0:GH])
            nc.vector.tensor_add(t1, t1, nn_[:, GH:2 * GH])
            n_sb = gtmp.tile([128, GH], f32, tag="n_sb")
            nc.scalar.activation(n_sb, t1, TANH)
            h4n = gtmp.tile([128, GH], f32, tag="h4")
            th_new = [gtmp.tile([128, 128], bf16, tag=f"th{half}",
                                name=f"th{half}")
                      for half in range(2)]
            for half in range(2):
                hs = slice(128 * half, 128 * half + 128)
                nc.vector.tensor_mul(h4n[:, hs], omz[:, hs], n_sb[:, hs])
                nc.vector.tensor_add(h4n[:, hs], h4n[:, hs], zh[:, hs])
                tpp = ps_tp.tile([128, 128], f32, tag="tp")
                nc.tensor.transpose(tpp, h4n[:, hs], ident)
                nc.vector.tensor_copy(th_new[half], tpp)
            return h4n, th_new

        # ---------------- warmup over previous poses ----------------
        # initial transpose of h0
        th = [gtmp.tile([128, 128], bf16, tag=f"th{half}", name=f"th{half}")
              for half in range(2)]
        for half in range(2):
            tpp = ps_tp.tile([128, 128], f32, tag="tp")
            nc.tensor.transpose(tpp, h4[:, 128 * half:128 * half + 128], ident)
            nc.vector.tensor_copy(th[half], tpp)

        for t in range(PLc):
            gi0 = posesT0[:, t * 32:(t + 1) * 32]
            gi1 = posesT1[:, t * 32:(t + 1) * 32]
            rz = ps_rz.tile([128, 512], f32, tag="rz")
            nn_ = ps_n.tile([128, 512], f32, tag="nn")
            emit_rz_whh(th, rz)
            emit_rz_gi(gi0, gi1, rz)
            emit_nn(th, gi0, gi1, nn_)
            h4, th = gru_tail(rz, nn_, h4)
            for _ in range(2):
                if chunks:
                    chunks.pop(0)()
        while chunks:
            chunks.pop(0)()

        # ---------------- main loop ----------------
        poseT0 = posesT0[:, (PLc - 1) * 32:PLc * 32]
        poseT1 = posesT1[:, (PLc - 1) * 32:PLc * 32]
        poses_dram = outs["poses"]

        # pre-issue step 0's gru matmuls (pipelined pattern)
        rz_cur = ps_rz.tile([128, 512], f32, tag="rz")
        nn_cur = ps_n.tile([128, 512], f32, tag="nn")
        emit_rz_whh(th, rz_cur)
        emit_nn_whh(th, nn_cur)
        emit_rz_gi(poseT0, poseT1, rz_cur)

        for t in range(T):
            emit_nn_gi(poseT0, poseT1, nn_cur)
            h4, th = gru_tail(rz_cur, nn_cur, h4)

            # scores (k-outer rounds for quadrant concurrency)
            sc = ps_sp.tile([128, 512], f32, tag="sp")
            for j in range(G):
                open_group(sc[32 * j:32 * j + 32, 0:2 * SC], 32, 32 * j)
            for k in range(8):
                lhsT = th_blk(th, k)
                for b in range(BL):
                    j, rr = b % 4, b // 4
                    mm(sc[32 * j:32 * j + 32, rr * SC:(rr + 1) * SC],
                       lhsT, projT[k][:, b * SC:(b + 1) * SC],
                       start=False, stop=(k == 7), tp=(0, 32 * j))

            # next step's W_hh rz+nn rounds: fills the PE during softmax
            if t < T - 1:
                rz_nxt = ps_rz.tile([128, 512], f32, tag="rz")
                nn_nxt = ps_n.tile([128, 512], f32, tag="nn")
                emit_rz_whh(th, rz_nxt)
                emit_nn_whh(th, nn_nxt)

            # softmax via sigmoid: exp(s - max) = 1/(1 - sig(s - max)) - 1
            nmax = gtmp.tile([128, 4], f32, tag="nmax")
            sig = gtmp.tile([128, 2 * SC], f32, tag="sig")
            rcp = gtmp.tile([128, 2 * SC], f32, tag="rcp")
            sumr = gtmp.tile([128, 4], f32, tag="sumr")
            rinv = gtmp.tile([128, 4], f32, tag="rinv")
            w_sb = gtmp.tile([128, 2 * SC], f32, tag="w_sb")
            # col layout in nmax/sumr/rinv: enc rr -> col rr, word rr -> 2+rr
            sc2 = sc[:, 0:2 * SC].rearrange("p (r c) -> p r c", r=2)
            nc.vector.reduce_max(out=nmax[:, 0:2], in_=sc2[:, :, 0:SCE],
                                 axis=X, negate=True)
            nc.vector.reduce_max(out=nmax[:, 2:4], in_=sc2[:, :, SCE:SC],
                                 axis=X, negate=True)
            for rr in range(2):
                nc.scalar.activation(sig[:, rr * SC:rr * SC + SCE],
                                     sc[:, rr * SC:rr * SC + SCE],
                                     SIG, bias=nmax[:, rr:rr + 1])
                nc.scalar.activation(sig[:, rr * SC + SCE:(rr + 1) * SC],
                                     sc[:, rr * SC + SCE:(rr + 1) * SC],
                                     SIG, bias=nmax[:, 2 + rr:3 + rr])
            nc.vector.tensor_scalar(rcp, sig, -1.0, 1.0, MULT, ADD)
            nc.vector.reciprocal_approx_fast(out=rcp, in_=rcp)
            rcp2 = rcp.rearrange("p (r c) -> p r c", r=2)
            nc.vector.reduce_sum(out=sumr[:, 0:2], in_=rcp2[:, :, 0:SCE],
                                 axis=X)
            nc.vector.reduce_sum(out=sumr[:, 2:4], in_=rcp2[:, :, SCE:SC],
                                 axis=X)
            nc.vector.tensor_scalar(rinv[:, 0:2], sumr[:, 0:2],
                                    float(SCE), None, SUB)
            nc.vector.tensor_scalar(rinv[:, 2:4], sumr[:, 2:4],
                                    float(SCW), None, SUB)
            nc.vector.reciprocal(rinv, rinv)
            nc.vector.tensor_scalar_sub(w_sb, rcp, 1.0)
            for rr in range(2):
                nc.vector.tensor_scalar_mul(w_sb[:, rr * SC:rr * SC + SCE],
                                            w_sb[:, rr * SC:rr * SC + SCE],
                                            rinv[:, rr:rr + 1])
            for rr in range(2):
                nc.vector.tensor_scalar_mul(
                    w_sb[:, rr * SC + SCE:(rr + 1) * SC],
                    w_sb[:, rr * SC + SCE:(rr + 1) * SC],
                    rinv[:, 2 + rr:3 + rr])

            # transpose softmax weights -> zero-padded bf16 stationaries
            for rr in range(2):
                tpp = ps_tp.tile([128, 128], f32, tag="tp")
                nc.tensor.transpose(tpp, w_sb[:, rr * SC:rr * SC + SCE], ident)
                nc.vector.tensor_copy(wte[rr][:, 4 * rr:4 * rr + 100:33],
                                      tpp[:, 4 * rr:4 * rr + 100:33])
                tp2 = ps_tp.tile([128, 128], f32, tag="tp")
                nc.tensor.transpose(tp2[0:64, :],
                                    w_sb[:, rr * SC + SCE:rr * SC + SC], ident)
                nc.vector.tensor_copy(wtw[rr][:, 4 * rr:4 * rr + 100:33],
                                      tp2[0:64, 4 * rr:4 * rr + 100:33])

            # pose accumulation: one [32,136] region per quadrant; both
            # rr-halves accumulate (invalid wte/wtw columns are zero)
            pp = ps_sp.tile([128, 512], f32, tag="sp")
            for j in range(G):
                open_group(pp[32 * j:32 * j + 32, 0:OP], 32, 32 * j)
            for k in range(8):
                lhsT = th_blk(th, k)
                for j in range(G):
                    mm(pp[32 * j:32 * j + 32, 0:OP], lhsT, woh[k],
                       start=False, stop=False, tp=(0, 32 * j))
            for rr in range(2):
                for j in range(G):
                    b = 4 * rr + j
                    mm(pp[32 * j:32 * j + 32, 0:OP],
                       wte[rr][:, 32 * j:32 * j + 32],
                       p2e[:, b * OP:(b + 1) * OP],
                       start=False, stop=False, tp=(0, 32 * j))
            for rr in range(2):
                for j in range(G):
                    b = 4 * rr + j
                    mm(pp[32 * j:32 * j + 32, 0:OP],
                       wtw[rr][0:64, 32 * j:32 * j + 32],
                       p2w[0:64, b * OP:(b + 1) * OP],
                       start=False, stop=(rr == 1), tp=(0, 32 * j))

            nc.vector.tensor_copy(pose_sb2[:, 0:O], pp[:, 0:O])
            for rr in range(2):
                nc.sync.dma_start(
                    out=poses_dram[t, 4 * rr:4 * rr + 4, :],
                    in_=pose_sb2[4 * rr:4 * rr + 100:33, 0:O],
                )
            if t == T - 1:
                break
            # pose -> transposed gi stationaries (valid cols only; rest
            # stay zero from the prologue memset)
            tpp = ps_tp.tile([128, 128], f32, tag="tp")
            nc.tensor.transpose(tpp, pose_sb2[:, 0:128], ident)
            tp2 = ps_tp.tile([128, 128], f32, tag="tp")
            nc.tensor.transpose(tp2[0:8, :], pose_sb2[:, 128:OP], ident)
            for rr in range(2):
                nc.vector.tensor_copy(pt0[:, 4 * rr:4 * rr + 4],
                                      tpp[:, 4 * rr:4 * rr + 100:33])
                nc.vector.tensor_copy(pt1[:, 4 * rr:4 * rr + 4],
                                      tp2[0:8, 4 * rr:4 * rr + 100:33])
            poseT0, poseT1 = pt0, pt1
            # finish next step's rz accumulation now that poseT is ready
            emit_rz_gi(poseT0, poseT1, rz_nxt)
            rz_cur, nn_cur = rz_nxt, nn_nxt


def _build(T, PLc):
    import concourse.tile as tile
    from concourse import bacc, mybir

    f32 = mybir.dt.float32
    bf16 = mybir.dt.bfloat16
    nc = bacc.Bacc("TRN2", target_bir_lowering=False, debug=False,
                   num_devices=NCORES)
    ins = {}

    def di(name, shape, dt=bf16):
        ins[name] = nc.dram_tensor(name, list(shape), dt,
                                   kind="ExternalInput").ap()

    di("xt_enc", (E + 1, BL * S))
    di("xt_word", (201, BL * WL))
    di("eht", (E + 1, 32))
    di("poses_t", (136, PLc * 32))
    di("whh_t", (H, 3 * H))
    di("gbias", (1, 3 * H))
    di("wih_t", (136, 3 * H))
    di("woh_t", (H, OP))
    di("woc_t", (H, OP))
    di("wow_t", (H, OP))
    di("bout", (1, OP))
    di("watt_t", (E + 1, H))
    di("wwatt_t", (201, H))
    di("wed_t", (E + 1, H))
    outs = {"poses": nc.dram_tensor("poses", [T, BL, O], f32,
                                    kind="ExternalOutput").ap()}
    with tile.TileContext(nc) as tc:
        _body(tc, outs, ins, T, PLc)
    nc.compile()
    return nc


def _host_prep(inputs, PLc=PL):
    """Per-core input maps (host transposes + weight prep), bf16."""
    import ml_dtypes
    bf = ml_dtypes.bfloat16

    enc = np.asarray(inputs["encoder_states"], np.float32)
    ehid = np.asarray(inputs["encoder_hidden"], np.float32)
    pp = np.asarray(inputs["previous_poses"], np.float32)
    words = np.asarray(inputs["words"], np.float32)
    W_ed, b_ed = np.asarray(inputs["W_ed"], np.float32), np.asarray(inputs["b_ed"], np.float32)
    W_att, b_att = np.asarray(inputs["W_att"], np.float32), np.asarray(inputs["b_att"], np.float32)
    W_watt, b_watt = np.asarray(inputs["W_watt"], np.float32), np.asarray(inputs["b_watt"], np.float32)
    W_ih, W_hh = np.asarray(inputs["W_ih"], np.float32), np.asarray(inputs["W_hh"], np.float32)
    b_ih, b_hh = np.asarray(inputs["b_ih"], np.float32), np.asarray(inputs["b_hh"], np.float32)
    W_out, b_out = np.asarray(inputs["W_out"], np.float32), np.asarray(inputs["b_out"], np.float32)

    gc = _group_cols()
    bihg = b_ih[gc]
    bhhg = b_hh[gc]
    # wih ones-lane bias: full (b_ih+b_hh) in rz slots, b_ih only in n slots
    wih_bias = bihg + bhhg
    # b_hh_n rides its own bias round (it is scaled by r)
    gnb = np.zeros(3 * H, np.float32)
    for j in range(G):
        c0 = j * 3 * GH
        wih_bias[c0 + 512:c0 + 768] = bihg[c0 + 512:c0 + 768]
        gnb[c0 + 512:c0 + 768] = bhhg[c0 + 512:c0 + 768]

    whh_t = W_hh.T[:, gc]
    wih_t = np.zeros((136, 3 * H), np.float32)
    wih_t[:O] = W_ih.T[:, gc]
    wih_t[O] = wih_bias

    woh_t = np.zeros((H, OP), np.float32)
    woh_t[:, :O] = W_out[:, :H].T
    woc_t = np.zeros((H, OP), np.float32)
    woc_t[:, :O] = W_out[:, H:2 * H].T
    wow_t = np.zeros((H, OP), np.float32)
    wow_t[:, :O] = W_out[:, 2 * H:].T
    bout = np.zeros((1, OP), np.float32)
    bout[0, :O] = b_out

    watt_t = np.concatenate([W_att.T, b_att[None, :]], 0)
    wwatt_t = np.concatenate([W_watt.T, b_watt[None, :]], 0)
    wed_t = np.concatenate([W_ed.T, b_ed[None, :]], 0)

    shared = dict(whh_t=whh_t, gbias=gnb[None, :], wih_t=wih_t, woh_t=woh_t,
                  woc_t=woc_t, wow_t=wow_t, bout=bout, watt_t=watt_t,
                  wwatt_t=wwatt_t, wed_t=wed_t)
    shared = {k: np.ascontiguousarray(v.astype(bf)) for k, v in shared.items()}

    in_maps = []
    for c in range(NCORES):
        bs = slice(c * BL, (c + 1) * BL)
        xt_enc = np.zeros((E + 1, BL * S), np.float32)
        xt_enc[:E] = np.transpose(enc[:, bs, :], (2, 1, 0)).reshape(E, BL * S)
        xt_enc[E] = 1.0
        xt_word = np.zeros((201, BL * WL), np.float32)
        xt_word[:200] = np.transpose(words[:, bs, :], (2, 1, 0)).reshape(200, BL * WL)
        xt_word[200] = 1.0
        eh = np.transpose(ehid[:, bs, :], (1, 0, 2)).reshape(BL, E)
        eht = np.zeros((E + 1, 32), np.float32)
        eht[:E, :BL] = eh.T
        eht[E, :BL] = 1.0
        poses_t = np.zeros((136, PLc, 32), np.float32)
        poses_t[:O, :, :BL] = np.transpose(pp[:, bs, :], (2, 0, 1))
        poses_t[O, :, :BL] = 1.0
        poses_t = poses_t.reshape(136, PLc * 32)
        m = dict(xt_enc=xt_enc, xt_word=xt_word, eht=eht, poses_t=poses_t)
        m = {k: np.ascontiguousarray(v.astype(bf)) for k, v in m.items()}
        m.update(shared)
        in_maps.append(m)
    return in_maps


def kernel(**inputs):
    from concourse.bass_utils import run_bass_kernel_spmd

    T = int(inputs["real_poses_len"])
    PLc = int(inputs["previous_poses"].shape[0])
    key = (T, PLc)
    if key not in _progs:
        _progs[key] = _build(T, PLc)
    nc = _progs[key]
    in_maps = _host_prep(inputs, PLc)
    trace = bool(int(os.environ.get("KERNEL_TRACE", "0")))
    res = run_bass_kernel_spmd(nc, in_maps, core_ids=list(range(NCORES)),
                               trace=trace)
    if trace:
        kernel.last_exec_time_ns = res.exec_time_ns
        kernel.last_mean_exec_time_ns = res.mean_exec_time_ns
    out = np.concatenate([res.results[c]["poses"] for c in range(NCORES)], axis=1)
    return out.astype(np.float32)


# revision 23
# speedup vs baseline: 1.0865x; 1.0513x over previous
"""Trainium2 Bass kernel for nn_Decoder (GRU decoder with dual attention).

Strategy (8 NeuronCores, batch-parallel, zero collectives):
  - Shard batch B=64 -> 8 per core; replicate all weights.
  - Matmul operands bf16; fp32 PSUM accumulation; gates/softmax/h fp32.
  - GRU gate matmuls: stationary = transposed hidden state (th blocks),
    moving = weights, 3H split into 4 PE column-group quadrants.
    Matmuls are issued in quadrant ROUNDS (same k-tile across all 4
    quadrants back-to-back) so the 4 quadrant streams run concurrently
    (PE matmul starts are pc-monotone; round order avoids cross-quadrant
    serialization).
  - Software pipelining: the next step's W_hh rz-rounds are issued right
    after this step's score matmuls, so the PE stays busy during the
    softmax (DVE/ACT) phase.
  - Softmax uses SIGMOID instead of EXP: exp(s-max) = 1/(1-sigmoid(s-max)) - 1,
    so the scalar engine never swaps activation tables (sig/tanh share one).
  - Pose accumulation: W_out folded into attention values (p2e/p2w);
    both batch-halves (rr=0/1) accumulate into ONE [32,136] psum region
    per quadrant; transposed softmax-weight tiles have all invalid
    columns zeroed (memset once; only valid columns rewritten per step).
  - Gate bias fully folded into the wih ones-lane (rz+gi_n parts); only
    b_hh_n needs its own tiny N=256 bias round (it multiplies r).
  - Outputs: 2 strided-partition DMAs per step (batches rr*4..rr*4+4).

Layouts:
  h4  [128, 256] fp32: row 32*j+b = h[b, j*256 : (j+1)*256], b<8 valid
  th[half] [128,128] bf16: th[half][k, 32*jj+b] = h[b, jj*256+half*128+k]
  projT[k] [128, 8*192] bf16: projT[k][kk, b*192+c]: c<128 enc proj s=c,
    c in 128:192 word proj wl=c-128; h-dim k*128+kk, batch b; biases folded.
  p2e [128, 8*136]: p2e[s, b*136+o] = (W_oc @ enc_proj[s,b] + b_out)[o]
  p2w [64, 8*136]: same for words with W_ow, no bias.
  score/pose psum rows: batch b lives at row 33*(b%4)+4*(b//4).
"""

import os
import sys

sys.path.insert(0, "/opt/trn_rl_repo")

import numpy as np

S, B, E, H, O, WL, PL = 128, 64, 1024, 1024, 135, 64, 32
NCORES = 8
BL = B // NCORES          # 8 batches per core
G = 4                     # PE column-group quadrants
GH = H // G               # 256 hidden dims per quadrant
OP = 136                  # padded pose dim (135 + ones col for gi bias)
SCE, SCW = S, WL
SC = SCE + SCW            # 192 score cols per batch

_progs = {}


def _group_cols():
    """Column permutation of the 3H gate dim into G groups of [r|z|n]."""
    cols = []
    for j in range(G):
        h0 = j * GH
        cols.extend(range(h0, h0 + GH))
        cols.extend(range(H + h0, H + h0 + GH))
        cols.extend(range(2 * H + h0, 2 * H + h0 + GH))
    return np.asarray(cols)


def _body(tc, outs, ins, T, PLc):
    """Tile kernel body. ins/outs: dicts of DRAM APs."""
    from concourse import mybir
    from concourse.masks import make_identity

    nc = tc.nc
    f32 = mybir.dt.float32
    bf16 = mybir.dt.bfloat16
    SIG = mybir.ActivationFunctionType.Sigmoid
    TANH = mybir.ActivationFunctionType.Tanh
    X = mybir.AxisListType.X
    MULT = mybir.AluOpType.mult
    ADD = mybir.AluOpType.add
    SUB = mybir.AluOpType.subtract

    def mm(out, lhsT, rhs, start, stop, tp=None):
        nc.tensor.matmul(out, lhsT, rhs, start=start, stop=stop,
                         tile_position=tp, skip_group_check=True)

    import contextlib
    ctx = contextlib.ExitStack()
    with ctx:
        wp = ctx.enter_context(tc.tile_pool(name="wp", bufs=1))
        work = ctx.enter_context(tc.tile_pool(name="work", bufs=2))
        gtmp = ctx.enter_context(tc.tile_pool(name="gtmp", bufs=2))
        ps_rz = ctx.enter_context(tc.tile_pool(name="ps_rz", bufs=2, space="PSUM"))
        ps_n = ctx.enter_context(tc.tile_pool(name="ps_n", bufs=1, space="PSUM"))
        ps_sp = ctx.enter_context(tc.tile_pool(name="ps_sp", bufs=3, space="PSUM"))
        ps_tp = ctx.enter_context(tc.tile_pool(name="ps_tp", bufs=2, space="PSUM"))

        # ---------------- persistent weights ----------------
        whh = []
        for k in range(8):
            t = wp.tile([128, 3 * H], bf16, tag=f"whh{k}")
            nc.sync.dma_start(out=t, in_=ins["whh_t"][k * 128:(k + 1) * 128, :])
            whh.append(t)
        gnb = wp.tile([1, 3 * H], bf16, tag="gnb")
        nc.sync.dma_start(out=gnb, in_=ins["gbias"][:, :])
        wih0 = wp.tile([128, 3 * H], bf16, tag="wih0")
        nc.sync.dma_start(out=wih0, in_=ins["wih_t"][0:128, :])
        wih1 = wp.tile([8, 3 * H], bf16, tag="wih1")
        nc.sync.dma_start(out=wih1, in_=ins["wih_t"][128:136, :])
        woh = []
        for k in range(8):
            t = wp.tile([128, OP], bf16, tag=f"woh{k}")
            nc.sync.dma_start(out=t, in_=ins["woh_t"][k * 128:(k + 1) * 128, :])
            woh.append(t)
        posesT0 = wp.tile([128, PLc * 32], bf16, tag="posesT0")
        nc.sync.dma_start(out=posesT0, in_=ins["poses_t"][0:128, :])
        posesT1 = wp.tile([8, PLc * 32], bf16, tag="posesT1")
        nc.sync.dma_start(out=posesT1, in_=ins["poses_t"][128:136, :])

        ident = wp.tile([128, 128], f32, tag="ident")
        make_identity(nc, ident[:, :])
        ones1 = wp.tile([1, 128], bf16, tag="ones1")
        nc.vector.memset(ones1, 1.0)
        zrow = wp.tile([1, 512], bf16, tag="zrow")
        nc.vector.memset(zrow, 0.0)

        def open_group(pr_region, m, base=0):
            # dummy start=True matmul on resident operands: clears the psum
            # region without inheriting DMA waits on the first real matmul
            mm(pr_region, ones1[:, 0:m], zrow[:, 0:pr_region.shape[-1]],
               start=True, stop=False, tp=(0, base))
        bout_sb = wp.tile([1, OP], bf16, tag="bout_sb")
        nc.sync.dma_start(out=bout_sb, in_=ins["bout"][:, :])

        projT = [wp.tile([128, BL * SC], bf16, tag=f"projT{m}", name=f"projT{m}")
                 for m in range(8)]
        p2e = wp.tile([128, BL * OP], bf16, tag="p2e")
        p2w = wp.tile([64, BL * OP], bf16, tag="p2w")

        # persistent per-step tiles: invalid lanes zeroed ONCE here, only
        # valid lanes rewritten inside the loop.
        wte = [wp.tile([128, 128], bf16, tag=f"wte{rr}", name=f"wte{rr}")
               for rr in range(2)]
        wtw = [wp.tile([64, 128], bf16, tag=f"wtw{rr}", name=f"wtw{rr}")
               for rr in range(2)]
        for rr in range(2):
            nc.vector.memset(wte[rr], 0.0)
            nc.vector.memset(wtw[rr], 0.0)
        pt0 = wp.tile([128, 32], bf16, tag="pt0")
        pt1 = wp.tile([8, 32], bf16, tag="pt1")
        nc.vector.memset(pt0, 0.0)
        nc.vector.memset(pt1, 0.0)
        pose_sb2 = wp.tile([128, OP], f32, tag="pose_sb")
        nc.vector.memset(pose_sb2[:, O:OP], 1.0)

        # ---------------- prologue: h0 ----------------
        ehk = []
        for k in range(8):
            t = wp.tile([128, 32], bf16, tag=f"ehk{k}")
            nc.sync.dma_start(out=t, in_=ins["eht"][k * 128:(k + 1) * 128, :])
            ehk.append(t)
        eh_ones = wp.tile([1, 32], bf16, tag="eh_ones")
        nc.sync.dma_start(out=eh_ones, in_=ins["eht"][1024:1025, :])

        h0p = ps_sp.tile([128, 512], f32, tag="sp")
        for j in range(G):
            open_group(h0p[32 * j:32 * j + 32, 0:GH], 32, 32 * j)
        for k in range(9):
            kp = 128 if k < 8 else 1
            lhsT = ehk[k] if k < 8 else eh_ones
            wed = work.tile([128, H], bf16, tag="wstream", bufs=9,
                            name=f"wed{k}")
            nc.sync.dma_start(out=wed[:kp, :],
                              in_=ins["wed_t"][k * 128:k * 128 + kp, :])
            for j in range(G):
                mm(h0p[32 * j:32 * j + 32, 0:GH], lhsT,
                   wed[:kp, j * GH:(j + 1) * GH],
                   start=False, stop=(k == 8), tp=(0, 32 * j))
        h4 = gtmp.tile([128, GH], f32, tag="h4")
        nc.vector.tensor_copy(h4, h0p[:, 0:GH])

        # ---------------- prologue proj work, chunked ----------------
        # Emitted interleaved with warmup GRU steps: the independent
        # projection matmuls fill the PE during each warmup tail, keeping
        # HAM warm and hiding the warmup chain latency.
        chunks = []
        store = {}

        def c_xe(q):
            xe = []
            for k in range(9):
                kp = 128 if k < 8 else 1
                t = work.tile([128, 256], bf16, tag="xe", bufs=36,
                              name=f"xe{q}_{k}")
                nc.sync.dma_start(
                    out=t[:kp, :],
                    in_=ins["xt_enc"][k * 128:k * 128 + kp,
                                      q * 256:(q + 1) * 256],
                )
                xe.append(t)
            store[("xe", q)] = xe

        def c_enc(q, m):
            xe = store[("xe", q)]
            pr = ps_sp.tile([128, 512], f32, tag="sp", name="pr_enc")
            open_group(pr[:, 0:256], 128)
            for k in range(9):
                kp = 128 if k < 8 else 1
                wa = work.tile([128, 128], bf16, tag="wa", bufs=16,
                               name="wa_enc")
                nc.sync.dma_start(
                    out=wa[:kp, :],
                    in_=ins["watt_t"][k * 128:k * 128 + kp,
                                      m * 128:(m + 1) * 128],
                )
                mm(pr[:, 0:256], wa[:kp, :], xe[k][:kp, :],
                   start=False, stop=(k == 8))
            dst = projT[m].rearrange("p (b c) -> p b c", b=BL)
            b0 = q * 2
            nc.vector.tensor_copy(
                dst[:, b0:b0 + 2, 0:SCE],
                pr[:, 0:256].rearrange("p (b c) -> p b c", b=2),
            )

        def c_xw():
            xw0 = work.tile([128, 512], bf16, tag="xw0", bufs=1, name="xw0")
            nc.sync.dma_start(out=xw0, in_=ins["xt_word"][0:128, :])
            xw1 = work.tile([73, 512], bf16, tag="xw1", bufs=1, name="xw1")
            nc.sync.dma_start(out=xw1, in_=ins["xt_word"][128:201, :])
            store["xw"] = (xw0, xw1)

        def c_word(m):
            xw0, xw1 = store["xw"]
            pr = ps_sp.tile([128, 512], f32, tag="sp", name="pr_word")
            open_group(pr[:, 0:512], 128)
            for k in range(2):
                kp = 128 if k == 0 else 73
                ww = work.tile([128, 128], bf16, tag="wa", bufs=16,
                               name="wa_word")
                nc.sync.dma_start(
                    out=ww[:kp, :],
                    in_=ins["wwatt_t"][k * 128:k * 128 + kp,
                                       m * 128:(m + 1) * 128],
                )
                mm(pr, ww[:kp, :], (xw0 if k == 0 else xw1)[:kp, :],
                   start=False, stop=(k == 1))
            dst = projT[m].rearrange("p (b c) -> p b c", b=BL)
            nc.vector.tensor_copy(
                dst[:, :, SCE:SC],
                pr.rearrange("p (b c) -> p b c", b=BL),
            )

        def c_wocw():
            wocw = [work.tile([128, OP], bf16, tag="wocw", bufs=16,
                              name=f"wocw{k}") for k in range(8)]
            for k in range(8):
                nc.sync.dma_start(out=wocw[k],
                                  in_=ins["woc_t"][k * 128:(k + 1) * 128, :])
            store["wocw"] = wocw

        def c_p2e(b):
            wocw = store["wocw"]
            pr = ps_sp.tile([128, 512], f32, tag="sp", name="pr_p2e")
            open_group(pr[:, 0:OP], 128)
            for k in range(8):
                mm(pr[:, 0:OP], projT[k][:, b * SC:b * SC + SCE], wocw[k],
                   start=False, stop=False)
            mm(pr[:, 0:OP], ones1, bout_sb, start=False, stop=True)
            nc.vector.tensor_copy(p2e[:, b * OP:(b + 1) * OP], pr[:, 0:OP])

        def c_woww():
            woww = [work.tile([128, OP], bf16, tag="wocw", bufs=16,
                              name=f"woww{k}") for k in range(8)]
            for k in range(8):
                nc.sync.dma_start(out=woww[k],
                                  in_=ins["wow_t"][k * 128:(k + 1) * 128, :])
            store["woww"] = woww

        def c_p2w(b):
            woww = store["woww"]
            pr = ps_sp.tile([128, 512], f32, tag="sp", name="pr_p2w")
            open_group(pr[0:64, 0:OP], 64)
            for k in range(8):
                mm(pr[0:64, 0:OP], projT[k][:, b * SC + SCE:b * SC + SC],
                   woww[k], start=False, stop=(k == 7))
            nc.vector.tensor_copy(p2w[:, b * OP:(b + 1) * OP], pr[0:64, 0:OP])

        import functools
        for q in range(4):
            chunks.append(functools.partial(c_xe, q))
            for m in range(8):
                chunks.append(functools.partial(c_enc, q, m))
        chunks.append(c_xw)
        for m in range(8):
            chunks.append(functools.partial(c_word, m))
        chunks.append(c_wocw)
        for b in range(BL):
            chunks.append(functools.partial(c_p2e, b))
        chunks.append(c_woww)
        for b in range(BL):
            chunks.append(functools.partial(c_p2w, b))

        # ---------------- recurrent machinery ----------------
        def th_blk(th, k):
            return th[k % 2][:, 32 * (k // 2):32 * (k // 2) + 32]

        def emit_rz_whh(th, rz):
            # 8 quadrant-rounds of N=512 rz matmuls (W_hh k-tiles)
            for k in range(8):
                lhsT = th_blk(th, k)
                for j in range(G):
                    c0 = j * 3 * GH
                    mm(rz[32 * j:32 * j + 32, :], lhsT, whh[k][:, c0:c0 + 512],
                       start=(k == 0), stop=False, tp=(0, 32 * j))

        def emit_rz_gi(gi0, gi1, rz):
            for kk, lhsT in ((0, gi0), (1, gi1)):
                wih = wih0 if kk == 0 else wih1
                for j in range(G):
                    c0 = j * 3 * GH
                    mm(rz[32 * j:32 * j + 32, :], lhsT, wih[:, c0:c0 + 512],
                       start=False, stop=(kk == 1), tp=(0, 32 * j))

        def emit_nn_whh(th, nn_):
            # bias round (b_hh_n must be scaled by r -> kept out of wih lane)
            for j in range(G):
                c0 = j * 3 * GH
                mm(nn_[32 * j:32 * j + 32, 0:GH], ones1[:, 0:32],
                   gnb[:, c0 + 512:c0 + 768], start=True, stop=False,
                   tp=(0, 32 * j))
            for k in range(8):
                lhsT = th_blk(th, k)
                for j in range(G):
                    c0 = j * 3 * GH
                    mm(nn_[32 * j:32 * j + 32, 0:GH], lhsT,
                       whh[k][:, c0 + 512:c0 + 768],
                       start=False, stop=False, tp=(0, 32 * j))

        def emit_nn_gi(gi0, gi1, nn_):
            for kk, lhsT in ((0, gi0), (1, gi1)):
                wih = wih0 if kk == 0 else wih1
                for j in range(G):
                    c0 = j * 3 * GH
                    mm(nn_[32 * j:32 * j + 32, GH:2 * GH], lhsT,
                       wih[:, c0 + 512:c0 + 768],
                       start=(kk == 0), stop=(kk == 1), tp=(0, 32 * j))

        def emit_nn(th, gi0, gi1, nn_):
            emit_nn_whh(th, nn_)
            emit_nn_gi(gi0, gi1, nn_)

        def gru_tail(rz, nn_, h4_prev):
            """sigmoid/tanh tail; returns (h4_new, th_new)."""
            srz = gtmp.tile([128, 512], f32, tag="srz")
            nc.scalar.activation(srz[:, 0:GH], rz[:, 0:GH], SIG)
            nc.scalar.activation(srz[:, GH:2 * GH], rz[:, GH:2 * GH], SIG)
            omz = gtmp.tile([128, GH], f32, tag="omz")
            nc.scalar.activation(omz, rz[:, GH:2 * GH], SIG, scale=-1.0)
            zh = gtmp.tile([128, GH], f32, tag="zh")
            nc.gpsimd.tensor_mul(zh, srz[:, GH:2 * GH], h4_prev)
            t1 = gtmp.tile([128, GH], f32, tag="t1")
            nc.vector.tensor_mul(t1, srz[:, 0:GH], nn_[:, 0:GH])
            nc.vector.tensor_add(t1, t1, nn_[:, GH:2 * GH])
            n_sb = gtmp.tile([128, GH], f32, tag="n_sb")
            nc.scalar.activation(n_sb, t1, TANH)
            h4n = gtmp.tile([128, GH], f32, tag="h4")
            th_new = [gtmp.tile([128, 128], bf16, tag=f"th{half}",
                                name=f"th{half}")
                      for half in range(2)]
            for half in range(2):
                hs = slice(128 * half, 128 * half + 128)
                nc.vector.tensor_mul(h4n[:, hs], omz[:, hs], n_sb[:, hs])
                nc.vector.tensor_add(h4n[:, hs], h4n[:, hs], zh[:, hs])
                tpp = ps_tp.tile([128, 128], f32, tag="tp")
                nc.tensor.transpose(tpp, h4n[:, hs], ident)
                nc.vector.tensor_copy(th_new[half], tpp)
            return h4n, th_new

        # ---------------- warmup over previous poses ----------------
        # initial transpose of h0
        th = [gtmp.tile([128, 128], bf16, tag=f"th{half}", name=f"th{half}")
              for half in range(2)]
        for half in range(2):
            tpp = ps_tp.tile([128, 128], f32, tag="tp")
            nc.tensor.transpose(tpp, h4[:, 128 * half:128 * half + 128], ident)
            nc.vector.tensor_copy(th[half], tpp)

        for t in range(PLc):
            gi0 = posesT0[:, t * 32:(t + 1) * 32]
            gi1 = posesT1[:, t * 32:(t + 1) * 32]
            rz = ps_rz.tile([128, 512], f32, tag="rz")
            nn_ = ps_n.tile([128, 512], f32, tag="nn")
            emit_rz_whh(th, rz)
            emit_rz_gi(gi0, gi1, rz)
            emit_nn(th, gi0, gi1, nn_)
            h4, th = gru_tail(rz, nn_, h4)
            for _ in range(2):
                if chunks:
                    chunks.pop(0)()
        while chunks:
            chunks.pop(0)()

        # ---------------- main loop ----------------
        poseT0 = posesT0[:, (PLc - 1) * 32:PLc * 32]
        poseT1 = posesT1[:, (PLc - 1) * 32:PLc * 32]
        poses_dram = outs["poses"]

        # pre-issue step 0's gru matmuls (pipelined pattern)
        rz_cur = ps_rz.tile([128, 512], f32, tag="rz")
        nn_cur = ps_n.tile([128, 512], f32, tag="nn")
        emit_rz_whh(th, rz_cur)
        emit_nn_whh(th, nn_cur)
        emit_rz_gi(poseT0, poseT1, rz_cur)

        for t in range(T):
            emit_nn_gi(poseT0, poseT1, nn_cur)
            h4, th = gru_tail(rz_cur, nn_cur, h4)

            # scores (k-outer rounds for quadrant concurrency)
            sc = ps_sp.tile([128, 512], f32, tag="sp")
            for j in range(G):
                open_group(sc[32 * j:32 * j + 32, 0:2 * SC], 32, 32 * j)
            for k in range(8):
                lhsT = th_blk(th, k)
                for b in range(BL):
                    j, rr = b % 4, b // 4
                    mm(sc[32 * j:32 * j + 32, rr * SC:(rr + 1) * SC],
                       lhsT, projT[k][:, b * SC:(b + 1) * SC],
                       start=False, stop=(k == 7), tp=(0, 32 * j))

            # next step's W_hh rz+nn rounds: fills the PE during softmax
            if t < T - 1:
                rz_nxt = ps_rz.tile([128, 512], f32, tag="rz")
                nn_nxt = ps_n.tile([128, 512], f32, tag="nn")
                emit_rz_whh(th, rz_nxt)
                emit_nn_whh(th, nn_nxt)

            # pose Woh.h rounds need only th: issue them now so they also
            # run during the softmax phase, ahead of the weight transposes
            pp = ps_sp.tile([128, 512], f32, tag="sp")
            for j in range(G):
                open_group(pp[32 * j:32 * j + 32, 0:OP], 32, 32 * j)
            for k in range(8):
                lhsT = th_blk(th, k)
                for j in range(G):
                    mm(pp[32 * j:32 * j + 32, 0:OP], lhsT, woh[k],
                       start=False, stop=False, tp=(0, 32 * j))

            # softmax via sigmoid: exp(s - max) = 1/(1 - sig(s - max)) - 1
            nmax = gtmp.tile([128, 4], f32, tag="nmax")
            sig = gtmp.tile([128, 2 * SC], f32, tag="sig")
            rcp = gtmp.tile([128, 2 * SC], f32, tag="rcp")
            sumr = gtmp.tile([128, 4], f32, tag="sumr")
            rinv = gtmp.tile([128, 4], f32, tag="rinv")
            w_sb = gtmp.tile([128, 2 * SC], f32, tag="w_sb")
            # col layout in nmax/sumr/rinv: enc rr -> col rr, word rr -> 2+rr
            sc2 = sc[:, 0:2 * SC].rearrange("p (r c) -> p r c", r=2)
            nc.vector.reduce_max(out=nmax[:, 0:2], in_=sc2[:, :, 0:SCE],
                                 axis=X, negate=True)
            nc.vector.reduce_max(out=nmax[:, 2:4], in_=sc2[:, :, SCE:SC],
                                 axis=X, negate=True)
            for rr in range(2):
                nc.scalar.activation(sig[:, rr * SC:rr * SC + SCE],
                                     sc[:, rr * SC:rr * SC + SCE],
                                     SIG, bias=nmax[:, rr:rr + 1])
                nc.scalar.activation(sig[:, rr * SC + SCE:(rr + 1) * SC],
                                     sc[:, rr * SC + SCE:(rr + 1) * SC],
                                     SIG, bias=nmax[:, 2 + rr:3 + rr])
            nc.vector.tensor_scalar(rcp, sig, -1.0, 1.0, MULT, ADD)
            nc.vector.reciprocal_approx_fast(out=rcp, in_=rcp)
            rcp2 = rcp.rearrange("p (r c) -> p r c", r=2)
            nc.vector.reduce_sum(out=sumr[:, 0:2], in_=rcp2[:, :, 0:SCE],
                                 axis=X)
            nc.vector.reduce_sum(out=sumr[:, 2:4], in_=rcp2[:, :, SCE:SC],
                                 axis=X)
            nc.vector.tensor_scalar(rinv[:, 0:2], sumr[:, 0:2],
                                    float(SCE), None, SUB)
            nc.vector.tensor_scalar(rinv[:, 2:4], sumr[:, 2:4],
                                    float(SCW), None, SUB)
            nc.vector.reciprocal(rinv, rinv)
            nc.vector.tensor_scalar_sub(w_sb, rcp, 1.0)
            for rr in range(2):
                nc.vector.tensor_scalar_mul(w_sb[:, rr * SC:rr * SC + SCE],
                                            w_sb[:, rr * SC:rr * SC + SCE],
                                            rinv[:, rr:rr + 1])
            for rr in range(2):
                nc.vector.tensor_scalar_mul(
                    w_sb[:, rr * SC + SCE:(rr + 1) * SC],
                    w_sb[:, rr * SC + SCE:(rr + 1) * SC],
                    rinv[:, 2 + rr:3 + rr])

            # transpose softmax weights -> zero-padded bf16 stationaries
            for rr in range(2):
                tpp = ps_tp.tile([128, 128], f32, tag="tp")
                nc.tensor.transpose(tpp, w_sb[:, rr * SC:rr * SC + SCE], ident)
                nc.vector.tensor_copy(wte[rr][:, 4 * rr:4 * rr + 100:33],
                                      tpp[:, 4 * rr:4 * rr + 100:33])
                tp2 = ps_tp.tile([128, 128], f32, tag="tp")
                nc.tensor.transpose(tp2[0:64, :],
                                    w_sb[:, rr * SC + SCE:rr * SC + SC], ident)
                nc.vector.tensor_copy(wtw[rr][:, 4 * rr:4 * rr + 100:33],
                                      tp2[0:64, 4 * rr:4 * rr + 100:33])

            # pose value folds: both rr-halves accumulate into the same
            # [32,136] region (invalid wte/wtw columns are zero)
            for rr in range(2):
                for j in range(G):
                    b = 4 * rr + j
                    mm(pp[32 * j:32 * j + 32, 0:OP],
                       wte[rr][:, 32 * j:32 * j + 32],
                       p2e[:, b * OP:(b + 1) * OP],
                       start=False, stop=False, tp=(0, 32 * j))
            for rr in range(2):
                for j in range(G):
                    b = 4 * rr + j
                    mm(pp[32 * j:32 * j + 32, 0:OP],
                       wtw[rr][0:64, 32 * j:32 * j + 32],
                       p2w[0:64, b * OP:(b + 1) * OP],
                       start=False, stop=(rr == 1), tp=(0, 32 * j))

            nc.vector.tensor_copy(pose_sb2[:, 0:O], pp[:, 0:O])
            for rr in range(2):
                nc.sync.dma_start(
                    out=poses_dram[t, 4 * rr:4 * rr + 4, :],
                    in_=pose_sb2[4 * rr:4 * rr + 100:33, 0:O],
                )
            if t == T - 1:
                break
            # pose -> transposed gi stationaries (valid cols only; rest
            # stay zero from the prologue memset)
            tpp = ps_tp.tile([128, 128], f32, tag="tp")
            nc.tensor.transpose(tpp, pose_sb2[:, 0:128], ident)
            tp2 = ps_tp.tile([128, 128], f32, tag="tp")
            nc.tensor.transpose(tp2[0:8, :], pose_sb2[:, 128:OP], ident)
            for rr in range(2):
                nc.vector.tensor_copy(pt0[:, 4 * rr:4 * rr + 4],
                                      tpp[:, 4 * rr:4 * rr + 100:33])
                nc.vector.tensor_copy(pt1[:, 4 * rr:4 * rr + 4],
                                      tp2[0:8, 4 * rr:4 * rr + 100:33])
            poseT0, poseT1 = pt0, pt1
            # finish next step's rz accumulation now that poseT is ready
            emit_rz_gi(poseT0, poseT1, rz_nxt)
            rz_cur, nn_cur = rz_nxt, nn_nxt


def _build(T, PLc):
    import concourse.tile as tile
    from concourse import bacc, mybir

    f32 = mybir.dt.float32
    bf16 = mybir.dt.bfloat16
    nc = bacc.Bacc("TRN2", target_bir_lowering=False, debug=False,
                   num_devices=NCORES)
    ins = {}

    def di(name, shape, dt=bf16):
        ins[name] = nc.dram_tensor(name, list(shape), dt,
                                   kind="ExternalInput").ap()

    di("xt_enc", (E + 1, BL * S))
    di("xt_word", (201, BL * WL))
    di("eht", (E + 1, 32))
    di("poses_t", (136, PLc * 32))
    di("whh_t", (H, 3 * H))
    di("gbias", (1, 3 * H))
    di("wih_t", (136, 3 * H))
    di("woh_t", (H, OP))
    di("woc_t", (H, OP))
    di("wow_t", (H, OP))
    di("bout", (1, OP))
    di("watt_t", (E + 1, H))
    di("wwatt_t", (201, H))
    di("wed_t", (E + 1, H))
    outs = {"poses": nc.dram_tensor("poses", [T, BL, O], f32,
                                    kind="ExternalOutput").ap()}
    with tile.TileContext(nc) as tc:
        _body(tc, outs, ins, T, PLc)
    nc.compile()
    return nc


def _host_prep(inputs, PLc=PL):
    """Per-core input maps (host transposes + weight prep), bf16."""
    import ml_dtypes
    bf = ml_dtypes.bfloat16

    enc = np.asarray(inputs["encoder_states"], np.float32)
    ehid = np.asarray(inputs["encoder_hidden"], np.float32)
    pp = np.asarray(inputs["previous_poses"], np.float32)
    words = np.asarray(inputs["words"], np.float32)
    W_ed, b_ed = np.asarray(inputs["W_ed"], np.float32), np.asarray(inputs["b_ed"], np.float32)
    W_att, b_att = np.asarray(inputs["W_att"], np.float32), np.asarray(inputs["b_att"], np.float32)
    W_watt, b_watt = np.asarray(inputs["W_watt"], np.float32), np.asarray(inputs["b_watt"], np.float32)
    W_ih, W_hh = np.asarray(inputs["W_ih"], np.float32), np.asarray(inputs["W_hh"], np.float32)
    b_ih, b_hh = np.asarray(inputs["b_ih"], np.float32), np.asarray(inputs["b_hh"], np.float32)
    W_out, b_out = np.asarray(inputs["W_out"], np.float32), np.asarray(inputs["b_out"], np.float32)

    gc = _group_cols()
    bihg = b_ih[gc]
    bhhg = b_hh[gc]
    # wih ones-lane bias: full (b_ih+b_hh) in rz slots, b_ih only in n slots
    wih_bias = bihg + bhhg
    # b_hh_n rides its own bias round (it is scaled by r)
    gnb = np.zeros(3 * H, np.float32)
    for j in range(G):
        c0 = j * 3 * GH
        wih_bias[c0 + 512:c0 + 768] = bihg[c0 + 512:c0 + 768]
        gnb[c0 + 512:c0 + 768] = bhhg[c0 + 512:c0 + 768]

    whh_t = W_hh.T[:, gc]
    wih_t = np.zeros((136, 3 * H), np.float32)
    wih_t[:O] = W_ih.T[:, gc]
    wih_t[O] = wih_bias

    woh_t = np.zeros((H, OP), np.float32)
    woh_t[:, :O] = W_out[:, :H].T
    woc_t = np.zeros((H, OP), np.float32)
    woc_t[:, :O] = W_out[:, H:2 * H].T
    wow_t = np.zeros((H, OP), np.float32)
    wow_t[:, :O] = W_out[:, 2 * H:].T
    bout = np.zeros((1, OP), np.float32)
    bout[0, :O] = b_out

    watt_t = np.concatenate([W_att.T, b_att[None, :]], 0)
    wwatt_t = np.concatenate([W_watt.T, b_watt[None, :]], 0)
    wed_t = np.concatenate([W_ed.T, b_ed[None, :]], 0)

    shared = dict(whh_t=whh_t, gbias=gnb[None, :], wih_t=wih_t, woh_t=woh_t,
                  woc_t=woc_t, wow_t=wow_t, bout=bout, watt_t=watt_t,
                  wwatt_t=wwatt_t, wed_t=wed_t)
    shared = {k: np.ascontiguousarray(v.astype(bf)) for k, v in shared.items()}

    in_maps = []
    for c in range(NCORES):
        bs = slice(c * BL, (c + 1) * BL)
        xt_enc = np.zeros((E + 1, BL * S), np.float32)
        xt_enc[:E] = np.transpose(enc[:, bs, :], (2, 1, 0)).reshape(E, BL * S)
        xt_enc[E] = 1.0
        xt_word = np.zeros((201, BL * WL), np.float32)
        xt_word[:200] = np.transpose(words[:, bs, :], (2, 1, 0)).reshape(200, BL * WL)
        xt_word[200] = 1.0
        eh = np.transpose(ehid[:, bs, :], (1, 0, 2)).reshape(BL, E)
        eht = np.zeros((E + 1, 32), np.float32)
        eht[:E, :BL] = eh.T
        eht[E, :BL] = 1.0
        poses_t = np.zeros((136, PLc, 32), np.float32)
        poses_t[:O, :, :BL] = np.transpose(pp[:, bs, :], (2, 0, 1))
        poses_t[O, :, :BL] = 1.0
        poses_t = poses_t.reshape(136, PLc * 32)
        m = dict(xt_enc=xt_enc, xt_word=xt_word, eht=eht, poses_t=poses_t)
        m = {k: np.ascontiguousarray(v.astype(bf)) for k, v in m.items()}
        m.update(shared)
        in_maps.append(m)
    return in_maps


def kernel(**inputs):
    from concourse.bass_utils import run_bass_kernel_spmd

    T = int(inputs["real_poses_len"])
    PLc = int(inputs["previous_poses"].shape[0])
    key = (T, PLc)
    if key not in _progs:
        _progs[key] = _build(T, PLc)
    nc = _progs[key]
    in_maps = _host_prep(inputs, PLc)
    trace = bool(int(os.environ.get("KERNEL_TRACE", "0")))
    res = run_bass_kernel_spmd(nc, in_maps, core_ids=list(range(NCORES)),
                               trace=trace)
    if trace:
        kernel.last_exec_time_ns = res.exec_time_ns
        kernel.last_mean_exec_time_ns = res.mean_exec_time_ns
    out = np.concatenate([res.results[c]["poses"] for c in range(NCORES)], axis=1)
    return out.astype(np.float32)


# revision 24
# speedup vs baseline: 1.1006x; 1.0130x over previous
"""Trainium2 Bass kernel for nn_Decoder (GRU decoder with dual attention).

Strategy (8 NeuronCores, batch-parallel, zero collectives):
  - Shard batch B=64 -> 8 per core; replicate all weights.
  - Matmul operands bf16; fp32 PSUM accumulation; gates/softmax/h fp32.
  - GRU gate matmuls: stationary = transposed hidden state (th blocks),
    moving = weights, 3H split into 4 PE column-group quadrants.
    Matmuls are issued in quadrant ROUNDS (same k-tile across all 4
    quadrants back-to-back) so the 4 quadrant streams run concurrently
    (PE matmul starts are pc-monotone; round order avoids cross-quadrant
    serialization).
  - Software pipelining: the next step's W_hh rz-rounds are issued right
    after this step's score matmuls, so the PE stays busy during the
    softmax (DVE/ACT) phase.
  - Softmax uses SIGMOID instead of EXP: exp(s-max) = 1/(1-sigmoid(s-max)) - 1,
    so the scalar engine never swaps activation tables (sig/tanh share one).
  - Pose accumulation: W_out folded into attention values (p2e/p2w);
    both batch-halves (rr=0/1) accumulate into ONE [32,136] psum region
    per quadrant; transposed softmax-weight tiles have all invalid
    columns zeroed (memset once; only valid columns rewritten per step).
  - Gate bias fully folded into the wih ones-lane (rz+gi_n parts); only
    b_hh_n needs its own tiny N=256 bias round (it multiplies r).
  - Outputs: 2 strided-partition DMAs per step (batches rr*4..rr*4+4).

Layouts:
  h4  [128, 256] fp32: row 32*j+b = h[b, j*256 : (j+1)*256], b<8 valid
  th[half] [128,128] bf16: th[half][k, 32*jj+b] = h[b, jj*256+half*128+k]
  projT[k] [128, 8*192] bf16: projT[k][kk, b*192+c]: c<128 enc proj s=c,
    c in 128:192 word proj wl=c-128; h-dim k*128+kk, batch b; biases folded.
  p2e [128, 8*136]: p2e[s, b*136+o] = (W_oc @ enc_proj[s,b] + b_out)[o]
  p2w [64, 8*136]: same for words with W_ow, no bias.
  score/pose psum rows: batch b lives at row 33*(b%4)+4*(b//4).
"""

import os
import sys

sys.path.insert(0, "/opt/trn_rl_repo")

import numpy as np

S, B, E, H, O, WL, PL = 128, 64, 1024, 1024, 135, 64, 32
NCORES = 8
BL = B // NCORES          # 8 batches per core
G = 4                     # PE column-group quadrants
GH = H // G               # 256 hidden dims per quadrant
OP = 136                  # padded pose dim (135 + ones col for gi bias)
SCE, SCW = S, WL
SC = SCE + SCW            # 192 score cols per batch

_progs = {}


def _group_cols():
    """Column permutation of the 3H gate dim into G groups of [r|z|n]."""
    cols = []
    for j in range(G):
        h0 = j * GH
        cols.extend(range(h0, h0 + GH))
        cols.extend(range(H + h0, H + h0 + GH))
        cols.extend(range(2 * H + h0, 2 * H + h0 + GH))
    return np.asarray(cols)


def _body(tc, outs, ins, T, PLc):
    """Tile kernel body. ins/outs: dicts of DRAM APs."""
    from concourse import mybir
    from concourse.masks import make_identity

    nc = tc.nc
    f32 = mybir.dt.float32
    bf16 = mybir.dt.bfloat16
    SIG = mybir.ActivationFunctionType.Sigmoid
    TANH = mybir.ActivationFunctionType.Tanh
    X = mybir.AxisListType.X
    MULT = mybir.AluOpType.mult
    ADD = mybir.AluOpType.add
    SUB = mybir.AluOpType.subtract

    def mm(out, lhsT, rhs, start, stop, tp=None):
        nc.tensor.matmul(out, lhsT, rhs, start=start, stop=stop,
                         tile_position=tp, skip_group_check=True)

    import contextlib
    ctx = contextlib.ExitStack()
    with ctx:
        wp = ctx.enter_context(tc.tile_pool(name="wp", bufs=1))
        work = ctx.enter_context(tc.tile_pool(name="work", bufs=2))
        gtmp = ctx.enter_context(tc.tile_pool(name="gtmp", bufs=2))
        ps_rz = ctx.enter_context(tc.tile_pool(name="ps_rz", bufs=2, space="PSUM"))
        ps_n = ctx.enter_context(tc.tile_pool(name="ps_n", bufs=1, space="PSUM"))
        ps_sp = ctx.enter_context(tc.tile_pool(name="ps_sp", bufs=3, space="PSUM"))
        ps_tp = ctx.enter_context(tc.tile_pool(name="ps_tp", bufs=2, space="PSUM"))

        # ---------------- persistent weights ----------------
        whh = []
        for k in range(8):
            t = wp.tile([128, 3 * H], bf16, tag=f"whh{k}")
            nc.sync.dma_start(out=t, in_=ins["whh_t"][k * 128:(k + 1) * 128, :])
            whh.append(t)
        gnb = wp.tile([1, 3 * H], bf16, tag="gnb")
        nc.sync.dma_start(out=gnb, in_=ins["gbias"][:, :])
        wih0 = wp.tile([128, 3 * H], bf16, tag="wih0")
        nc.sync.dma_start(out=wih0, in_=ins["wih_t"][0:128, :])
        wih1 = wp.tile([8, 3 * H], bf16, tag="wih1")
        nc.sync.dma_start(out=wih1, in_=ins["wih_t"][128:136, :])
        woh = []
        for k in range(8):
            t = wp.tile([128, OP], bf16, tag=f"woh{k}")
            nc.sync.dma_start(out=t, in_=ins["woh_t"][k * 128:(k + 1) * 128, :])
            woh.append(t)
        posesT0 = wp.tile([128, PLc * 32], bf16, tag="posesT0")
        nc.sync.dma_start(out=posesT0, in_=ins["poses_t"][0:128, :])
        posesT1 = wp.tile([8, PLc * 32], bf16, tag="posesT1")
        nc.sync.dma_start(out=posesT1, in_=ins["poses_t"][128:136, :])

        ident = wp.tile([128, 128], f32, tag="ident")
        make_identity(nc, ident[:, :])
        ones1 = wp.tile([1, 128], bf16, tag="ones1")
        nc.vector.memset(ones1, 1.0)
        zrow = wp.tile([1, 512], bf16, tag="zrow")
        nc.vector.memset(zrow, 0.0)

        def open_group(pr_region, m, base=0):
            # dummy start=True matmul on resident operands: clears the psum
            # region without inheriting DMA waits on the first real matmul
            mm(pr_region, ones1[:, 0:m], zrow[:, 0:pr_region.shape[-1]],
               start=True, stop=False, tp=(0, base))
        bout_sb = wp.tile([1, OP], bf16, tag="bout_sb")
        nc.sync.dma_start(out=bout_sb, in_=ins["bout"][:, :])

        projT = [wp.tile([128, BL * SC], bf16, tag=f"projT{m}", name=f"projT{m}")
                 for m in range(8)]
        p2e = wp.tile([128, BL * OP], bf16, tag="p2e")
        p2w = wp.tile([64, BL * OP], bf16, tag="p2w")

        # persistent per-step tiles: invalid lanes zeroed ONCE here, only
        # valid lanes rewritten inside the loop.
        wte = [wp.tile([128, 128], bf16, tag=f"wte{rr}", name=f"wte{rr}")
               for rr in range(2)]
        wtw = [wp.tile([64, 128], bf16, tag=f"wtw{rr}", name=f"wtw{rr}")
               for rr in range(2)]
        for rr in range(2):
            nc.vector.memset(wte[rr], 0.0)
            nc.vector.memset(wtw[rr], 0.0)
        pt0 = wp.tile([128, 32], bf16, tag="pt0")
        pt1 = wp.tile([8, 32], bf16, tag="pt1")
        nc.vector.memset(pt0, 0.0)
        nc.vector.memset(pt1, 0.0)
        pose_sb2 = wp.tile([128, OP], f32, tag="pose_sb")
        nc.vector.memset(pose_sb2[:, O:OP], 1.0)

        # ---------------- prologue: h0 ----------------
        ehk = []
        for k in range(8):
            t = wp.tile([128, 32], bf16, tag=f"ehk{k}")
            nc.sync.dma_start(out=t, in_=ins["eht"][k * 128:(k + 1) * 128, :])
            ehk.append(t)
        eh_ones = wp.tile([1, 32], bf16, tag="eh_ones")
        nc.sync.dma_start(out=eh_ones, in_=ins["eht"][1024:1025, :])

        h0p = ps_sp.tile([128, 512], f32, tag="sp")
        for j in range(G):
            open_group(h0p[32 * j:32 * j + 32, 0:GH], 32, 32 * j)
        for k in range(9):
            kp = 128 if k < 8 else 1
            lhsT = ehk[k] if k < 8 else eh_ones
            wed = work.tile([128, H], bf16, tag="wstream", bufs=9,
                            name=f"wed{k}")
            nc.sync.dma_start(out=wed[:kp, :],
                              in_=ins["wed_t"][k * 128:k * 128 + kp, :])
            for j in range(G):
                mm(h0p[32 * j:32 * j + 32, 0:GH], lhsT,
                   wed[:kp, j * GH:(j + 1) * GH],
                   start=False, stop=(k == 8), tp=(0, 32 * j))
        h4 = gtmp.tile([128, GH], f32, tag="h4")
        nc.vector.tensor_copy(h4, h0p[:, 0:GH])

        # ---------------- prologue proj work, chunked ----------------
        # Emitted interleaved with warmup GRU steps: the independent
        # projection matmuls fill the PE during each warmup tail, keeping
        # HAM warm and hiding the warmup chain latency.
        chunks = []
        store = {}

        def c_xe(q):
            xe = []
            for k in range(9):
                kp = 128 if k < 8 else 1
                t = work.tile([128, 256], bf16, tag="xe", bufs=36,
                              name=f"xe{q}_{k}")
                nc.sync.dma_start(
                    out=t[:kp, :],
                    in_=ins["xt_enc"][k * 128:k * 128 + kp,
                                      q * 256:(q + 1) * 256],
                )
                xe.append(t)
            store[("xe", q)] = xe

        def c_enc(q, m):
            xe = store[("xe", q)]
            pr = ps_sp.tile([128, 512], f32, tag="sp", name="pr_enc")
            open_group(pr[:, 0:256], 128)
            for k in range(9):
                kp = 128 if k < 8 else 1
                wa = work.tile([128, 128], bf16, tag="wa", bufs=16,
                               name="wa_enc")
                nc.sync.dma_start(
                    out=wa[:kp, :],
                    in_=ins["watt_t"][k * 128:k * 128 + kp,
                                      m * 128:(m + 1) * 128],
                )
                mm(pr[:, 0:256], wa[:kp, :], xe[k][:kp, :],
                   start=False, stop=(k == 8))
            dst = projT[m].rearrange("p (b c) -> p b c", b=BL)
            b0 = q * 2
            nc.vector.tensor_copy(
                dst[:, b0:b0 + 2, 0:SCE],
                pr[:, 0:256].rearrange("p (b c) -> p b c", b=2),
            )

        def c_xw():
            xw0 = work.tile([128, 512], bf16, tag="xw0", bufs=1, name="xw0")
            nc.sync.dma_start(out=xw0, in_=ins["xt_word"][0:128, :])
            xw1 = work.tile([73, 512], bf16, tag="xw1", bufs=1, name="xw1")
            nc.sync.dma_start(out=xw1, in_=ins["xt_word"][128:201, :])
            store["xw"] = (xw0, xw1)

        def c_word(m):
            xw0, xw1 = store["xw"]
            pr = ps_sp.tile([128, 512], f32, tag="sp", name="pr_word")
            open_group(pr[:, 0:512], 128)
            for k in range(2):
                kp = 128 if k == 0 else 73
                ww = work.tile([128, 128], bf16, tag="wa", bufs=16,
                               name="wa_word")
                nc.sync.dma_start(
                    out=ww[:kp, :],
                    in_=ins["wwatt_t"][k * 128:k * 128 + kp,
                                       m * 128:(m + 1) * 128],
                )
                mm(pr, ww[:kp, :], (xw0 if k == 0 else xw1)[:kp, :],
                   start=False, stop=(k == 1))
            dst = projT[m].rearrange("p (b c) -> p b c", b=BL)
            nc.vector.tensor_copy(
                dst[:, :, SCE:SC],
                pr.rearrange("p (b c) -> p b c", b=BL),
            )

        def c_wocw():
            wocw = [work.tile([128, OP], bf16, tag="wocw", bufs=16,
                              name=f"wocw{k}") for k in range(8)]
            for k in range(8):
                nc.sync.dma_start(out=wocw[k],
                                  in_=ins["woc_t"][k * 128:(k + 1) * 128, :])
            store["wocw"] = wocw

        def c_p2e(b):
            wocw = store["wocw"]
            pr = ps_sp.tile([128, 512], f32, tag="sp", name="pr_p2e")
            open_group(pr[:, 0:OP], 128)
            for k in range(8):
                mm(pr[:, 0:OP], projT[k][:, b * SC:b * SC + SCE], wocw[k],
                   start=False, stop=False)
            mm(pr[:, 0:OP], ones1, bout_sb, start=False, stop=True)
            nc.vector.tensor_copy(p2e[:, b * OP:(b + 1) * OP], pr[:, 0:OP])

        def c_woww():
            woww = [work.tile([128, OP], bf16, tag="wocw", bufs=16,
                              name=f"woww{k}") for k in range(8)]
            for k in range(8):
                nc.sync.dma_start(out=woww[k],
                                  in_=ins["wow_t"][k * 128:(k + 1) * 128, :])
            store["woww"] = woww

        def c_p2w(b):
            woww = store["woww"]
            pr = ps_sp.tile([128, 512], f32, tag="sp", name="pr_p2w")
            open_group(pr[0:64, 0:OP], 64)
            for k in range(8):
                mm(pr[0:64, 0:OP], projT[k][:, b * SC + SCE:b * SC + SC],
                   woww[k], start=False, stop=(k == 7))
            nc.vector.tensor_copy(p2w[:, b * OP:(b + 1) * OP], pr[0:64, 0:OP])

        import functools
        for q in range(4):
            chunks.append(functools.partial(c_xe, q))
            for m in range(8):
                chunks.append(functools.partial(c_enc, q, m))
        chunks.append(c_xw)
        for m in range(8):
            chunks.append(functools.partial(c_word, m))
        chunks.append(c_wocw)
        for b in range(BL):
            chunks.append(functools.partial(c_p2e, b))
        chunks.append(c_woww)
        for b in range(BL):
            chunks.append(functools.partial(c_p2w, b))

        # ---------------- recurrent machinery ----------------
        def th_blk(th, k):
            return th[k % 2][:, 32 * (k // 2):32 * (k // 2) + 32]

        def emit_rz_whh(th, rz):
            # 8 quadrant-rounds of N=512 rz matmuls (W_hh k-tiles)
            for k in range(8):
                lhsT = th_blk(th, k)
                for j in range(G):
                    c0 = j * 3 * GH
                    mm(rz[32 * j:32 * j + 32, :], lhsT, whh[k][:, c0:c0 + 512],
                       start=(k == 0), stop=False, tp=(0, 32 * j))

        def emit_rz_gi(gi0, gi1, rz):
            for kk, lhsT in ((0, gi0), (1, gi1)):
                wih = wih0 if kk == 0 else wih1
                for j in range(G):
                    c0 = j * 3 * GH
                    mm(rz[32 * j:32 * j + 32, :], lhsT, wih[:, c0:c0 + 512],
                       start=False, stop=(kk == 1), tp=(0, 32 * j))

        def emit_nn_whh(th, nn_):
            # bias round (b_hh_n must be scaled by r -> kept out of wih lane)
            for j in range(G):
                c0 = j * 3 * GH
                mm(nn_[32 * j:32 * j + 32, 0:GH], ones1[:, 0:32],
                   gnb[:, c0 + 512:c0 + 768], start=True, stop=False,
                   tp=(0, 32 * j))
            for k in range(8):
                lhsT = th_blk(th, k)
                for j in range(G):
                    c0 = j * 3 * GH
                    mm(nn_[32 * j:32 * j + 32, 0:GH], lhsT,
                       whh[k][:, c0 + 512:c0 + 768],
                       start=False, stop=False, tp=(0, 32 * j))

        def emit_nn_gi(gi0, gi1, nn_):
            for kk, lhsT in ((0, gi0), (1, gi1)):
                wih = wih0 if kk == 0 else wih1
                for j in range(G):
                    c0 = j * 3 * GH
                    mm(nn_[32 * j:32 * j + 32, GH:2 * GH], lhsT,
                       wih[:, c0 + 512:c0 + 768],
                       start=(kk == 0), stop=(kk == 1), tp=(0, 32 * j))

        def emit_nn(th, gi0, gi1, nn_):
            emit_nn_whh(th, nn_)
            emit_nn_gi(gi0, gi1, nn_)

        def gru_tail(rz, nn_, h4_prev):
            """sigmoid/tanh tail; returns (h4_new, th_new)."""
            srz = gtmp.tile([128, 512], f32, tag="srz")
            nc.scalar.activation(srz[:, 0:GH], rz[:, 0:GH], SIG)
            nc.scalar.activation(srz[:, GH:2 * GH], rz[:, GH:2 * GH], SIG)
            omz = gtmp.tile([128, GH], f32, tag="omz")
            nc.scalar.activation(omz, rz[:, GH:2 * GH], SIG, scale=-1.0)
            zh = gtmp.tile([128, GH], f32, tag="zh")
            nc.gpsimd.tensor_mul(zh, srz[:, GH:2 * GH], h4_prev)
            t1 = gtmp.tile([128, GH], f32, tag="t1")
            nc.vector.tensor_mul(t1, srz[:, 0:GH], nn_[:, 0:GH])
            nc.vector.tensor_add(t1, t1, nn_[:, GH:2 * GH])
            n_sb = gtmp.tile([128, GH], f32, tag="n_sb")
            nc.scalar.activation(n_sb, t1, TANH)
            h4n = gtmp.tile([128, GH], f32, tag="h4")
            th_new = [gtmp.tile([128, 128], bf16, tag=f"th{half}",
                                name=f"th{half}")
                      for half in range(2)]
            for half in range(2):
                hs = slice(128 * half, 128 * half + 128)
                nc.vector.tensor_mul(h4n[:, hs], omz[:, hs], n_sb[:, hs])
                nc.vector.tensor_add(h4n[:, hs], h4n[:, hs], zh[:, hs])
                tpp = ps_tp.tile([128, 128], f32, tag="tp")
                nc.tensor.transpose(tpp, h4n[:, hs], ident)
                nc.vector.tensor_copy(th_new[half], tpp)
            return h4n, th_new

        # ---------------- warmup over previous poses ----------------
        # initial transpose of h0
        th = [gtmp.tile([128, 128], bf16, tag=f"th{half}", name=f"th{half}")
              for half in range(2)]
        for half in range(2):
            tpp = ps_tp.tile([128, 128], f32, tag="tp")
            nc.tensor.transpose(tpp, h4[:, 128 * half:128 * half + 128], ident)
            nc.vector.tensor_copy(th[half], tpp)

        for t in range(PLc):
            gi0 = posesT0[:, t * 32:(t + 1) * 32]
            gi1 = posesT1[:, t * 32:(t + 1) * 32]
            rz = ps_rz.tile([128, 512], f32, tag="rz")
            nn_ = ps_n.tile([128, 512], f32, tag="nn")
            emit_rz_whh(th, rz)
            emit_rz_gi(gi0, gi1, rz)
            emit_nn(th, gi0, gi1, nn_)
            h4, th = gru_tail(rz, nn_, h4)
            for _ in range(2):
                if chunks:
                    chunks.pop(0)()
        while chunks:
            chunks.pop(0)()

        # ---------------- main loop ----------------
        poseT0 = posesT0[:, (PLc - 1) * 32:PLc * 32]
        poseT1 = posesT1[:, (PLc - 1) * 32:PLc * 32]
        poses_dram = outs["poses"]

        # pre-issue step 0's gru matmuls (pipelined pattern)
        rz_cur = ps_rz.tile([128, 512], f32, tag="rz")
        nn_cur = ps_n.tile([128, 512], f32, tag="nn")
        emit_rz_whh(th, rz_cur)
        emit_nn_whh(th, nn_cur)
        emit_rz_gi(poseT0, poseT1, rz_cur)

        for t in range(T):
            emit_nn_gi(poseT0, poseT1, nn_cur)
            h4, th = gru_tail(rz_cur, nn_cur, h4)

            # scores (k-outer rounds for quadrant concurrency)
            sc = ps_sp.tile([128, 512], f32, tag="sp")
            for j in range(G):
                open_group(sc[32 * j:32 * j + 32, 0:2 * SC], 32, 32 * j)
            for k in range(8):
                lhsT = th_blk(th, k)
                for b in range(BL):
                    j, rr = b % 4, b // 4
                    mm(sc[32 * j:32 * j + 32, rr * SC:(rr + 1) * SC],
                       lhsT, projT[k][:, b * SC:(b + 1) * SC],
                       start=False, stop=(k == 7), tp=(0, 32 * j))

            # next step's W_hh rz+nn rounds: fills the PE during softmax
            if t < T - 1:
                rz_nxt = ps_rz.tile([128, 512], f32, tag="rz")
                nn_nxt = ps_n.tile([128, 512], f32, tag="nn")
                emit_rz_whh(th, rz_nxt)
                emit_nn_whh(th, nn_nxt)

            # pose Woh.h rounds need only th: issue them now so they also
            # run during the softmax phase, ahead of the weight transposes
            pp = ps_sp.tile([128, 512], f32, tag="sp")
            for j in range(G):
                open_group(pp[32 * j:32 * j + 32, 0:OP], 32, 32 * j)
            for k in range(8):
                lhsT = th_blk(th, k)
                for j in range(G):
                    mm(pp[32 * j:32 * j + 32, 0:OP], lhsT, woh[k],
                       start=False, stop=False, tp=(0, 32 * j))

            # softmax via sigmoid: exp(s - max) = 1/(1 - sig(s - max)) - 1
            nmax = gtmp.tile([128, 4], f32, tag="nmax")
            sig = gtmp.tile([128, 2 * SC], f32, tag="sig")
            rcp = gtmp.tile([128, 2 * SC], f32, tag="rcp")
            sumr = gtmp.tile([128, 4], f32, tag="sumr")
            rinv = gtmp.tile([128, 4], f32, tag="rinv")
            w_sb = gtmp.tile([128, 2 * SC], f32, tag="w_sb")
            # col layout in nmax/sumr/rinv: enc rr -> col rr, word rr -> 2+rr
            # DVE chain split per rr-half so rr0 pipelines under rr1's sigmoids
            sc2 = sc[:, 0:2 * SC].rearrange("p (r c) -> p r c", r=2)
            nc.vector.reduce_max(out=nmax[:, 0:2], in_=sc2[:, :, 0:SCE],
                                 axis=X, negate=True)
            nc.vector.reduce_max(out=nmax[:, 2:4], in_=sc2[:, :, SCE:SC],
                                 axis=X, negate=True)
            for rr in range(2):
                cs = rr * SC
                nc.scalar.activation(sig[:, cs:cs + SCE], sc[:, cs:cs + SCE],
                                     SIG, bias=nmax[:, rr:rr + 1])
                nc.scalar.activation(sig[:, cs + SCE:cs + SC],
                                     sc[:, cs + SCE:cs + SC],
                                     SIG, bias=nmax[:, 2 + rr:3 + rr])
                nc.vector.tensor_scalar(rcp[:, cs:cs + SC], sig[:, cs:cs + SC],
                                        -1.0, 1.0, MULT, ADD)
                nc.vector.reciprocal_approx_fast(out=rcp[:, cs:cs + SC],
                                                 in_=rcp[:, cs:cs + SC])
                nc.vector.tensor_scalar_sub(w_sb[:, cs:cs + SC],
                                            rcp[:, cs:cs + SC], 1.0)
                nc.vector.reduce_sum(out=sumr[:, rr:rr + 1],
                                     in_=rcp[:, cs:cs + SCE], axis=X)
                nc.vector.reduce_sum(out=sumr[:, 2 + rr:3 + rr],
                                     in_=rcp[:, cs + SCE:cs + SC], axis=X)
                nc.vector.tensor_scalar(rinv[:, rr:rr + 1],
                                        sumr[:, rr:rr + 1],
                                        float(SCE), None, SUB)
                nc.vector.tensor_scalar(rinv[:, 2 + rr:3 + rr],
                                        sumr[:, 2 + rr:3 + rr],
                                        float(SCW), None, SUB)
                nc.vector.reciprocal(rinv[:, rr:4:2], rinv[:, rr:4:2])
                nc.vector.tensor_scalar_mul(w_sb[:, cs:cs + SCE],
                                            w_sb[:, cs:cs + SCE],
                                            rinv[:, rr:rr + 1])
                nc.vector.tensor_scalar_mul(w_sb[:, cs + SCE:cs + SC],
                                            w_sb[:, cs + SCE:cs + SC],
                                            rinv[:, 2 + rr:3 + rr])

            # transpose softmax weights -> zero-padded bf16 stationaries
            for rr in range(2):
                tpp = ps_tp.tile([128, 128], f32, tag="tp")
                nc.tensor.transpose(tpp, w_sb[:, rr * SC:rr * SC + SCE], ident)
                nc.vector.tensor_copy(wte[rr][:, 4 * rr:4 * rr + 100:33],
                                      tpp[:, 4 * rr:4 * rr + 100:33])
                tp2 = ps_tp.tile([128, 128], f32, tag="tp")
                nc.tensor.transpose(tp2[0:64, :],
                                    w_sb[:, rr * SC + SCE:rr * SC + SC], ident)
                nc.vector.tensor_copy(wtw[rr][:, 4 * rr:4 * rr + 100:33],
                                      tp2[0:64, 4 * rr:4 * rr + 100:33])

            # pose value folds: both rr-halves accumulate into the same
            # [32,136] region (invalid wte/wtw columns are zero)
            for rr in range(2):
                for j in range(G):
                    b = 4 * rr + j
                    mm(pp[32 * j:32 * j + 32, 0:OP],
                       wte[rr][:, 32 * j:32 * j + 32],
                       p2e[:, b * OP:(b + 1) * OP],
                       start=False, stop=False, tp=(0, 32 * j))
            for rr in range(2):
                for j in range(G):
                    b = 4 * rr + j
                    mm(pp[32 * j:32 * j + 32, 0:OP],
                       wtw[rr][0:64, 32 * j:32 * j + 32],
                       p2w[0:64, b * OP:(b + 1) * OP],
                       start=False, stop=(rr == 1), tp=(0, 32 * j))

            nc.vector.tensor_copy(pose_sb2[:, 0:O], pp[:, 0:O])
            for rr in range(2):
                nc.sync.dma_start(
                    out=poses_dram[t, 4 * rr:4 * rr + 4, :],
                    in_=pose_sb2[4 * rr:4 * rr + 100:33, 0:O],
                )
            if t == T - 1:
                break
            # pose -> transposed gi stationaries (valid cols only; rest
            # stay zero from the prologue memset)
            tpp = ps_tp.tile([128, 128], f32, tag="tp")
            nc.tensor.transpose(tpp, pose_sb2[:, 0:128], ident)
            tp2 = ps_tp.tile([128, 128], f32, tag="tp")
            nc.tensor.transpose(tp2[0:8, :], pose_sb2[:, 128:OP], ident)
            for rr in range(2):
                nc.vector.tensor_copy(pt0[:, 4 * rr:4 * rr + 4],
                                      tpp[:, 4 * rr:4 * rr + 100:33])
                nc.vector.tensor_copy(pt1[:, 4 * rr:4 * rr + 4],
                                      tp2[0:8, 4 * rr:4 * rr + 100:33])
            poseT0, poseT1 = pt0, pt1
            # finish next step's rz accumulation now that poseT is ready
            emit_rz_gi(poseT0, poseT1, rz_nxt)
            rz_cur, nn_cur = rz_nxt, nn_nxt


def _build(T, PLc):
    import concourse.tile as tile
    from concourse import bacc, mybir

    f32 = mybir.dt.float32
    bf16 = mybir.dt.bfloat16
    nc = bacc.Bacc("TRN2", target_bir_lowering=False, debug=False,
                   num_devices=NCORES)
    ins = {}

    def di(name, shape, dt=bf16):
        ins[name] = nc.dram_tensor(name, list(shape), dt,
                                   kind="ExternalInput").ap()

    di("xt_enc", (E + 1, BL * S))
    di("xt_word", (201, BL * WL))
    di("eht", (E + 1, 32))
    di("poses_t", (136, PLc * 32))
    di("whh_t", (H, 3 * H))
    di("gbias", (1, 3 * H))
    di("wih_t", (136, 3 * H))
    di("woh_t", (H, OP))
    di("woc_t", (H, OP))
    di("wow_t", (H, OP))
    di("bout", (1, OP))
    di("watt_t", (E + 1, H))
    di("wwatt_t", (201, H))
    di("wed_t", (E + 1, H))
    outs = {"poses": nc.dram_tensor("poses", [T, BL, O], f32,
                                    kind="ExternalOutput").ap()}
    with tile.TileContext(nc) as tc:
        _body(tc, outs, ins, T, PLc)
    nc.compile()
    return nc


def _host_prep(inputs, PLc=PL):
    """Per-core input maps (host transposes + weight prep), bf16."""
    import ml_dtypes
    bf = ml_dtypes.bfloat16

    enc = np.asarray(inputs["encoder_states"], np.float32)
    ehid = np.asarray(inputs["encoder_hidden"], np.float32)
    pp = np.asarray(inputs["previous_poses"], np.float32)
    words = np.asarray(inputs["words"], np.float32)
    W_ed, b_ed = np.asarray(inputs["W_ed"], np.float32), np.asarray(inputs["b_ed"], np.float32)
    W_att, b_att = np.asarray(inputs["W_att"], np.float32), np.asarray(inputs["b_att"], np.float32)
    W_watt, b_watt = np.asarray(inputs["W_watt"], np.float32), np.asarray(inputs["b_watt"], np.float32)
    W_ih, W_hh = np.asarray(inputs["W_ih"], np.float32), np.asarray(inputs["W_hh"], np.float32)
    b_ih, b_hh = np.asarray(inputs["b_ih"], np.float32), np.asarray(inputs["b_hh"], np.float32)
    W_out, b_out = np.asarray(inputs["W_out"], np.float32), np.asarray(inputs["b_out"], np.float32)

    gc = _group_cols()
    bihg = b_ih[gc]
    bhhg = b_hh[gc]
    # wih ones-lane bias: full (b_ih+b_hh) in rz slots, b_ih only in n slots
    wih_bias = bihg + bhhg
    # b_hh_n rides its own bias round (it is scaled by r)
    gnb = np.zeros(3 * H, np.float32)
    for j in range(G):
        c0 = j * 3 * GH
        wih_bias[c0 + 512:c0 + 768] = bihg[c0 + 512:c0 + 768]
        gnb[c0 + 512:c0 + 768] = bhhg[c0 + 512:c0 + 768]

    whh_t = W_hh.T[:, gc]
    wih_t = np.zeros((136, 3 * H), np.float32)
    wih_t[:O] = W_ih.T[:, gc]
    wih_t[O] = wih_bias

    woh_t = np.zeros((H, OP), np.float32)
    woh_t[:, :O] = W_out[:, :H].T
    woc_t = np.zeros((H, OP), np.float32)
    woc_t[:, :O] = W_out[:, H:2 * H].T
    wow_t = np.zeros((H, OP), np.float32)
    wow_t[:, :O] = W_out[:, 2 * H:].T
    bout = np.zeros((1, OP), np.float32)
    bout[0, :O] = b_out

    watt_t = np.concatenate([W_att.T, b_att[None, :]], 0)
    wwatt_t = np.concatenate([W_watt.T, b_watt[None, :]], 0)
    wed_t = np.concatenate([W_ed.T, b_ed[None, :]], 0)

    shared = dict(whh_t=whh_t, gbias=gnb[None, :], wih_t=wih_t, woh_t=woh_t,
                  woc_t=woc_t, wow_t=wow_t, bout=bout, watt_t=watt_t,
                  wwatt_t=wwatt_t, wed_t=wed_t)
    shared = {k: np.ascontiguousarray(v.astype(bf)) for k, v in shared.items()}

    in_maps = []
    for c in range(NCORES):
        bs = slice(c * BL, (c + 1) * BL)
        xt_enc = np.zeros((E + 1, BL * S), np.float32)
        xt_enc[:E] = np.transpose(enc[:, bs, :], (2, 1, 0)).reshape(E, BL * S)
        xt_enc[E] = 1.0
        xt_word = np.zeros((201, BL * WL), np.float32)
        xt_word[:200] = np.transpose(words[:, bs, :], (2, 1, 0)).reshape(200, BL * WL)
        xt_word[200] = 1.0
        eh = np.transpose(ehid[:, bs, :], (1, 0, 2)).reshape(BL, E)
        eht = np.zeros((E + 1, 32), np.float32)
        eht[:E, :BL] = eh.T
        eht[E, :BL] = 1.0
        poses_t = np.zeros((136, PLc, 32), np.float32)
        poses_t[:O, :, :BL] = np.transpose(pp[:, bs, :], (2, 0, 1))
        poses_t[O, :, :BL] = 1.0
        poses_t = poses_t.reshape(136, PLc * 32)
        m = dict(xt_enc=xt_enc, xt_word=xt_word, eht=eht, poses_t=poses_t)
        m = {k: np.ascontiguousarray(v.astype(bf)) for k, v in m.items()}
        m.update(shared)
        in_maps.append(m)
    return in_maps


def kernel(**inputs):
    from concourse.bass_utils import run_bass_kernel_spmd

    T = int(inputs["real_poses_len"])
    PLc = int(inputs["previous_poses"].shape[0])
    key = (T, PLc)
    if key not in _progs:
        _progs[key] = _build(T, PLc)
    nc = _progs[key]
    in_maps = _host_prep(inputs, PLc)
    trace = bool(int(os.environ.get("KERNEL_TRACE", "0")))
    res = run_bass_kernel_spmd(nc, in_maps, core_ids=list(range(NCORES)),
                               trace=trace)
    if trace:
        kernel.last_exec_time_ns = res.exec_time_ns
        kernel.last_mean_exec_time_ns = res.mean_exec_time_ns
    out = np.concatenate([res.results[c]["poses"] for c in range(NCORES)], axis=1)
    return out.astype(np.float32)
